# Initial kernel scaffold
#
"""Trainium2 Bass kernel for nn_BitGenModel (BitNet-style dense transformer).

Sharding: data-parallel over batch (B=8) across 8 NeuronCores; each core runs
the full 2-layer transformer + final 32000-vocab projection for its batch
element. Weights are pre-quantized (ternary, bf16) and pre-transposed on the
host; activation quantization (per-tensor absmax int8) runs on device with
batch-local scales.

Numerics notes:
- BitLinear integer matmuls run on the PE in bf16: |qx|<=127 and qW in
  {-1,0,1} are exactly representable, accumulation is fp32 in PSUM => exact.
- Attention (scores, softmax, A@V) runs in fp32. Softmax uses per-key-row
  max subtraction with an exp(m_k) correction folded into the V operand,
  which is mathematically exact and needs no cross-partition reductions.
- LayerNorm uses DVE bn_stats/bn_aggr (fp32, free-dim reduction).
"""
import math
import os

import numpy as np
import ml_dtypes

import concourse.bass as bass
import concourse.mybir as mybir
import concourse.tile as tile
from concourse import bacc
from concourse.bass_utils import run_bass_kernel_spmd

F32 = mybir.dt.float32
BF16 = mybir.dt.bfloat16
I8 = mybir.dt.int8
AX = mybir.AxisListType
ALU = mybir.AluOpType
ACTF = mybir.ActivationFunctionType

B, T, S = 8, 512, 256
D, H, HD, FF, V, L = 512, 8, 64, 2048, 32000, 2
EPS = 1e-5
P = 128
TB = T // P            # 4 token blocks
DB = D // P            # 4 feature blocks
FB = FF // P           # 16 ffn blocks
VN = 512               # vocab tile width
VBF = V // VN          # 62 full vocab blocks
VREM = V - VBF * VN    # 256 remainder
NCORES = 8
INV_SQRT_HD = 1.0 / math.sqrt(HD)

_NC_CACHE = {}


def _quantize_weight(W):
    """Host-side eval-mode BitNet weight quantization (matches reference)."""
    W = np.asarray(W, dtype=np.float32)
    w_scale = np.maximum(np.abs(W).mean(dtype=np.float32), np.float32(1e-5))
    qW = np.sign(W) * (np.abs(W) > np.float32(0.5) * w_scale)
    qWT = np.ascontiguousarray(qW.astype(np.float32).T.astype(ml_dtypes.bfloat16))
    return qWT, float(w_scale)


def _col_layout(v):
    """[D] -> [DBLK, 128, 1] so b[j] is a contiguous [128,1] per-partition col."""
    v = np.asarray(v, dtype=np.float32)
    n = v.shape[0]
    return np.ascontiguousarray(v.reshape(n // P, P, 1))


def build(wscales, debug=False):
    nc = bacc.Bacc("TRN2", target_bir_lowering=False, debug=False,
                   num_devices=NCORES)

    # ---------------- I/O declarations ----------------
    x_in = nc.dram_tensor("x", [T, D], F32, kind="ExternalInput")
    enc_in = nc.dram_tensor("enc", [B * S, D], F32, kind="ExternalInput")
    tri_in = nc.dram_tensor("tri", [P, P], F32, kind="ExternalInput")

    wq, wk, wv, wo, wenc, w1, w2 = [], [], [], [], [], [], []
    bq, bk, bv, b1 = [], [], [], []
    for l in range(L):
        wq.append(nc.dram_tensor(f"wq{l}", [D, D], BF16, kind="ExternalInput"))
        wk.append(nc.dram_tensor(f"wk{l}", [D, D], BF16, kind="ExternalInput"))
        wv.append(nc.dram_tensor(f"wv{l}", [D, D], BF16, kind="ExternalInput"))
        wo.append(nc.dram_tensor(f"wo{l}", [D, D], BF16, kind="ExternalInput"))
        wenc.append(nc.dram_tensor(f"wenc{l}", [D, D], BF16, kind="ExternalInput"))
        w1.append(nc.dram_tensor(f"w1{l}", [D, FF], BF16, kind="ExternalInput"))
        w2.append(nc.dram_tensor(f"w2{l}", [FF, D], BF16, kind="ExternalInput"))
        bq.append(nc.dram_tensor(f"bq{l}", [DB, P, 1], F32, kind="ExternalInput"))
        bk.append(nc.dram_tensor(f"bk{l}", [DB, P, 1], F32, kind="ExternalInput"))
        bv.append(nc.dram_tensor(f"bv{l}", [DB, P, 1], F32, kind="ExternalInput"))
        b1.append(nc.dram_tensor(f"b1{l}", [FB, P, 1], F32, kind="ExternalInput"))
    wout_in = nc.dram_tensor("wout", [D, V], BF16, kind="ExternalInput")

    out = nc.dram_tensor("logits", [T, V], F32, kind="ExternalOutput")

    dbg = {}
    if debug:
        for name, shape in [("dbg_h0", [T, D]), ("dbg_h1", [T, D]),
                            ("dbg_ln1", [T, D]), ("dbg_attn", [D, T]),
                            ("dbg_qT", [D, T]), ("dbg_gelu", [FF, T])]:
            dbg[name] = nc.dram_tensor(name, shape, F32, kind="ExternalOutput")

    with tile.TileContext(nc) as tc:
        _body(nc, tc, wscales, x_in, enc_in, tri_in,
              wq, wk, wv, wo, wenc, w1, w2, bq, bk, bv, b1, wout_in, out, dbg)
    nc.compile()
    return nc


def _body(nc, tc, ws, x_in, enc_in, tri_in,
          wq, wk, wv, wo, wenc, w1, w2, bq_t, bk_t, bv_t, b1_t, wout_in, out,
          dbg):
    from contextlib import ExitStack
    ctx = ExitStack()
    with ctx:
        # ---------------- pools ----------------
        hp = ctx.enter_context(tc.tile_pool(name="hp", bufs=TB))          # resident h
        const = ctx.enter_context(tc.tile_pool(name="const", bufs=1))
        lnp = ctx.enter_context(tc.tile_pool(name="lnp", bufs=TB + 1))    # ln outputs
        stat = ctx.enter_context(tc.tile_pool(name="stat", bufs=4))       # bn stats etc
        colp = ctx.enter_context(tc.tile_pool(name="colp", bufs=6))       # [128,1] cols
        qxp = ctx.enter_context(tc.tile_pool(name="qxp", bufs=3))         # int8 temps
        qtp = ctx.enter_context(tc.tile_pool(name="qtp", bufs=2 * DB))    # bf16 transposed acts
        wsp = ctx.enter_context(tc.tile_pool(name="wsp", bufs=2))         # small weights
        wfp = ctx.enter_context(tc.tile_pool(name="wfp", bufs=2))         # W1/W2 tiles
        qkp = ctx.enter_context(tc.tile_pool(name="qkp", bufs=2 * DB))    # q/k f32
        vp = ctx.enter_context(tc.tile_pool(name="vp", bufs=TB))          # v (+ones) f32
        vhp = ctx.enter_context(tc.tile_pool(name="vhp", bufs=3))         # scaled v head
        expp = ctx.enter_context(tc.tile_pool(name="expp", bufs=6))       # exp tiles f32
        atp = ctx.enter_context(tc.tile_pool(name="atp", bufs=TB))        # attn out f32
        gelp = ctx.enter_context(tc.tile_pool(name="gelp", bufs=FB))      # gelu f32
        qgp = ctx.enter_context(tc.tile_pool(name="qgp", bufs=FB))        # qgelu bf16
        drp = ctx.enter_context(tc.tile_pool(name="drp", bufs=4, space="DRAM"))
        outp = ctx.enter_context(tc.tile_pool(name="outp", bufs=4))       # logits stage
        wop = ctx.enter_context(tc.tile_pool(name="wop", bufs=3 * DB))    # wout stream
        ps = ctx.enter_context(tc.tile_pool(name="ps", bufs=6, space="PSUM"))
        pso = ctx.enter_context(tc.tile_pool(name="pso", bufs=2, space="PSUM"))

        # ---------------- constants ----------------
        tri = const.tile([P, P], F32)
        nc.sync.dma_start(out=tri[:], in_=tri_in[:])
        eps_c = const.tile([P, 1], F32)
        nc.vector.memset(eps_c[:], EPS)
        ones_row = const.tile([1, P], F32)
        nc.vector.memset(ones_row[:], 1.0)
        ident_bf = const.tile([P, P], BF16)
        from concourse.masks import make_identity
        make_identity(nc, ident_bf)

        # ---------------- load x into resident h tiles ----------------
        h = []
        for tb in range(TB):
            ht = hp.tile([P, D], F32, tag=f"h{tb}")
            nc.sync.dma_start(out=ht[:], in_=x_in[tb * P:(tb + 1) * P, :])
            h.append(ht)

        # =========== helpers ===========
        def cross_part_absmax(cols, tag):
            """cols: list of [128,1] f32 per-partition absmax -> [128,1] bcast
            of max over everything (batch-local), clamped at 1e-8."""
            acc = cols[0]
            for c in cols[1:]:
                nxt = colp.tile([P, 1], F32, tag=f"cpm_{tag}")
                nc.vector.tensor_max(out=nxt[:], in0=acc[:], in1=c[:])
                acc = nxt
            dcol = drp.tile([P, 1], F32, tag="dcol")
            nc.sync.dma_start(out=dcol[:], in_=acc[:])
            row = stat.tile([1, P], F32, tag="cprow")
            nc.sync.dma_start(out=row[:], in_=dcol[:].rearrange("a b -> b a"))
            m1 = stat.tile([1, 1], F32, tag="cpm1")
            nc.vector.tensor_reduce(out=m1[:], in_=row[:], axis=AX.X, op=ALU.max)
            mb = colp.tile([P, 1], F32, tag=f"mb_{tag}")
            nc.gpsimd.partition_broadcast(mb[:], m1[:], channels=P)
            mc = colp.tile([P, 1], F32, tag=f"mc_{tag}")
            nc.vector.tensor_scalar_max(out=mc[:], in0=mb[:], scalar1=1e-8)
            return mc

        def iscale_of(mhat, tag):
            isc = colp.tile([P, 1], F32, tag=f"isc_{tag}")
            nc.vector.tensor_scalar(out=isc[:], in0=mhat[:], scalar1=127.0,
                                    scalar2=None, op0=ALU.divide)
            return isc

        def quantize_tiles(src_tiles, isc, tag, n=D):
            """f32 [128,n] tiles -> bf16 [128,n] tiles (round+saturate int8)."""
            res = []
            for i, st in enumerate(src_tiles):
                q8 = qxp.tile([P, n], I8, tag=f"q8_{tag}")
                nc.vector.tensor_scalar(out=q8[:], in0=st[:], scalar1=isc[:],
                                        scalar2=None, op0=ALU.divide)
                qb = qtp.tile([P, n], BF16, tag=f"qb_{tag}{i}")
                nc.scalar.copy(out=qb[:], in_=q8[:])
                res.append(qb)
            return res

        def transpose_to(qtiles, tag):
            """qtiles: TB x [128, D] bf16 (rows=t) -> DB x [128, T] bf16 (rows=d)."""
            outt = [qtp.tile([P, T], BF16, tag=f"tr_{tag}{db}") for db in range(DB)]
            for tb in range(TB):
                for db in range(DB):
                    pst = ps.tile([P, VN], F32, tag="psum")
                    nc.tensor.transpose(pst[:P, :P],
                                        qtiles[tb][:, db * P:(db + 1) * P],
                                        ident_bf[:])
                    nc.vector.tensor_copy(out=outt[db][:, tb * P:(tb + 1) * P],
                                          in_=pst[:P, :P])
            return outt

        def layer_norm(tiles, gvec, bvec, apply_affine, tag):
            """Natural-layout layernorm over free dim; returns new tiles."""
            outs = []
            for tb in range(TB):
                st6 = stat.tile([P, 6], F32, tag="bn6")
                nc.vector.bn_stats(out=st6[:], in_=tiles[tb][:])
                mv = stat.tile([P, 2], F32, tag="bn2")
                nc.vector.bn_aggr(out=mv[:], in_=st6[:])
                std = colp.tile([P, 1], F32, tag="std")
                nc.scalar.activation(out=std[:], in_=mv[:, 1:2], func=ACTF.Sqrt,
                                     bias=eps_c[:], scale=1.0)
                rstd = colp.tile([P, 1], F32, tag="rstd")
                nc.vector.reciprocal(out=rstd[:], in_=std[:])
                ot = lnp.tile([P, D], F32, tag=f"ln_{tag}")
                nc.vector.tensor_scalar(out=ot[:], in0=tiles[tb][:],
                                        scalar1=mv[:, 0:1], scalar2=rstd[:],
                                        op0=ALU.subtract, op1=ALU.mult)
                if apply_affine:
                    gb = const.tile([P, D], F32, tag=f"g_{tag}")
                    bb = const.tile([P, D], F32, tag=f"b_{tag}")
                    nc.sync.dma_start(out=gb[:], in_=gvec)
                    nc.sync.dma_start(out=bb[:], in_=bvec)
                    nc.vector.tensor_mul(out=ot[:], in0=ot[:], in1=gb[:])
                    nc.vector.tensor_add(out=ot[:], in0=ot[:], in1=bb[:])
                outs.append(ot)
            return outs

        def absmax_cols(tiles, tag):
            cols = []
            for i, tl in enumerate(tiles):
                c = colp.tile([P, 1], F32, tag=f"am_{tag}")
                nc.vector.tensor_reduce(out=c[:], in_=tl[:], axis=AX.X,
                                        op=ALU.max, apply_absolute_value=True)
                cols.append(c)
            return cols

        # ---------------- ctx prep (encoder mean + quantize, once) ----------
        # mean over S per batch via ones-matmul (fp32), all 8 batches
        encf = const.tile([1, P], F32)      # f32 ones col as lhsT [128,1]
        ones_col = const.tile([P, 1], F32)
        nc.vector.memset(ones_col[:], 1.0)
        ctx_sb = const.tile([B, D], F32)    # ctx rows for all batches
        for b in range(B):
            pctx = ps.tile([P, VN], F32, tag="psum")
            for sb in range(S // P):
                et = lnp.tile([P, D], F32, tag="enc_ld")
                nc.sync.dma_start(
                    out=et[:], in_=enc_in[b * S + sb * P: b * S + (sb + 1) * P, :])
                nc.tensor.matmul(pctx[:1, :D], ones_col[:], et[:],
                                 start=(sb == 0), stop=(sb == S // P - 1))
            nc.scalar.activation(out=ctx_sb[b:b + 1, :], in_=pctx[:1, :D],
                                 func=ACTF.Copy, scale=1.0 / S)
        # global absmax of ctx (over all batches -> exact global scale)
        cc = colp.tile([B, 1], F32, tag="ctxam")
        nc.vector.tensor_reduce(out=cc[:], in_=ctx_sb[:], axis=AX.X,
                                op=ALU.max, apply_absolute_value=True)
        dctx = drp.tile([B, 1], F32, tag="dctx")
        nc.sync.dma_start(out=dctx[:], in_=cc[:])
        crow = stat.tile([1, B], F32, tag="ctxrow")
        nc.sync.dma_start(out=crow[:], in_=dctx[:].rearrange("a b -> b a"))
        cm1 = stat.tile([1, 1], F32, tag="ctxm1")
        nc.vector.tensor_reduce(out=cm1[:], in_=crow[:], axis=AX.X, op=ALU.max)
        cmb = colp.tile([P, 1], F32, tag="ctxmb")
        nc.gpsimd.partition_broadcast(cmb[:], cm1[:], channels=P)
        cmc = colp.tile([P, 1], F32, tag="ctxmc")
        nc.vector.tensor_scalar_max(out=cmc[:], in0=cmb[:], scalar1=1e-8)
        isc_ctx = iscale_of(cmc, "ctx")
        # quantize own batch's ctx row and convert to column layout [128, DB]
        q8row = stat.tile([1, D], I8, tag="q8ctx")
        nc.vector.tensor_scalar(out=q8row[:], in0=ctx_sb[0:1, :], scalar1=isc_ctx[:1, :],
                                scalar2=None, op0=ALU.divide)
        # NOTE: per-core slice is handled on host by rolling enc rows so that
        # this core's batch lands at row 0.  See kernel().
        dctx8 = drp.tile([1, D], I8, tag="dctx8")
        nc.sync.dma_start(out=dctx8[:], in_=q8row[:])
        qctx8 = stat.tile([P, DB], I8, tag="qctxT8")
        nc.sync.dma_start(out=qctx8[:],
                          in_=dctx8[:].rearrange("one (j p) -> p (one j)", p=P))
        qctxT = const.tile([P, DB], BF16)
        nc.scalar.copy(out=qctxT[:], in_=qctx8[:])

        # ---------------- transformer layers ----------------
        for l in range(L):
            wsq, wsk, wsv, wso, wse, ws1, ws2 = ws[l]

            # --- load weights (bf16, pre-transposed [in, out]) ---
            wq_sb = [wsp.tile([P, D], BF16, tag=f"wq{db}") for db in range(DB)]
            wk_sb = [wsp.tile([P, D], BF16, tag=f"wk{db}") for db in range(DB)]
            wv_sb = [wsp.tile([P, D], BF16, tag=f"wv{db}") for db in range(DB)]
            wo_sb = [wsp.tile([P, D], BF16, tag=f"wo{db}") for db in range(DB)]
            we_sb = [wsp.tile([P, D], BF16, tag=f"we{db}") for db in range(DB)]
            for db in range(DB):
                nc.sync.dma_start(out=wq_sb[db][:], in_=wq[l][db * P:(db + 1) * P, :])
                nc.sync.dma_start(out=wk_sb[db][:], in_=wk[l][db * P:(db + 1) * P, :])
                nc.sync.dma_start(out=wv_sb[db][:], in_=wv[l][db * P:(db + 1) * P, :])
                nc.sync.dma_start(out=wo_sb[db][:], in_=wo[l][db * P:(db + 1) * P, :])
                nc.sync.dma_start(out=we_sb[db][:], in_=wenc[l][db * P:(db + 1) * P, :])
            bq_sb = const.tile([DB, P, 1], F32, tag="bqc")
            bk_sb = const.tile([DB, P, 1], F32, tag="bkc")
            bv_sb = const.tile([DB, P, 1], F32, tag="bvc")
            nc.sync.dma_start(out=bq_sb[:], in_=bq_t[l][:])
            nc.sync.dma_start(out=bk_sb[:], in_=bk_t[l][:])
            nc.sync.dma_start(out=bv_sb[:], in_=bv_t[l][:])

            # --- ln1 + quantize + transpose ---
            ln1 = layer_norm(h, None, None, False, "ln1")
            if dbg and l == 0:
                for tb in range(TB):
                    nc.sync.dma_start(out=dbg["dbg_ln1"][tb * P:(tb + 1) * P, :],
                                      in_=ln1[tb][:])
            m_ln1 = cross_part_absmax(absmax_cols(ln1, "ln1"), f"ln1_{l}")
            isc1 = iscale_of(m_ln1, f"ln1_{l}")
            q1 = quantize_tiles(ln1, isc1, "ln1")
            q1T = transpose_to(q1, "ln1")     # DB x [128, T] bf16 (rows=d)

            # --- qkv matmuls ---
            # q/k: Option Q -> [o, t] transposed, f32 with dequant+bias fused
            sc_q = colp.tile([P, 1], F32, tag="sc_q")
            nc.scalar.mul(out=sc_q[:], in_=m_ln1[:], mul=wsq * INV_SQRT_HD / 127.0)
            sc_k = colp.tile([P, 1], F32, tag="sc_k")
            nc.scalar.mul(out=sc_k[:], in_=m_ln1[:], mul=wsk / 127.0)
            sc_v = colp.tile([P, 1], F32, tag="sc_v")
            nc.scalar.mul(out=sc_v[:], in_=m_ln1[:], mul=wsv / 127.0)
            qT, kT = [], []
            for ob in range(DB):
                pq = ps.tile([P, VN], F32, tag="psum")
                pk = ps.tile([P, VN], F32, tag="psum")
                for db in range(DB):
                    nc.tensor.matmul(pq[:, :T], wq_sb[db][:, ob * P:(ob + 1) * P],
                                     q1T[db][:], start=(db == 0), stop=(db == DB - 1))
                for db in range(DB):
                    nc.tensor.matmul(pk[:, :T], wk_sb[db][:, ob * P:(ob + 1) * P],
                                     q1T[db][:], start=(db == 0), stop=(db == DB - 1))
                qf = qkp.tile([P, T], F32, tag=f"qT{ob}")
                nc.scalar.activation(out=qf[:], in_=pq[:, :T], func=ACTF.Identity,
                                     bias=bq_sb[ob][:], scale=sc_q[:])
                kf = qkp.tile([P, T], F32, tag=f"kT{ob}")
                nc.scalar.activation(out=kf[:], in_=pk[:, :T], func=ACTF.Identity,
                                     bias=bk_sb[ob][:], scale=sc_k[:])
                qT.append(qf)
                kT.append(kf)
            if dbg and l == 0:
                for ob in range(DB):
                    nc.sync.dma_start(out=dbg["dbg_qT"][ob * P:(ob + 1) * P, :],
                                      in_=qT[ob][:])
            # v: Option P -> natural [t, o], packed [128, H, HD+1] with exp(m) col
            v_sb = []
            for tb in range(TB):
                pv = ps.tile([P, VN], F32, tag="psum")
                for db in range(DB):
                    nc.tensor.matmul(pv[:, :D], q1T[db][:, tb * P:(tb + 1) * P],
                                     wv_sb[db][:], start=(db == 0), stop=(db == DB - 1))
                vt = vp.tile([P, H, HD + 1], F32, tag=f"v{tb}")
                nc.scalar.activation(out=vt[:, :, :HD].rearrange("p h d -> p (h d)"),
                                     in_=pv[:, :D], func=ACTF.Identity,
                                     bias=bv_sb_flat(nc, bv_sb, colp), scale=sc_v[:])
                v_sb.append(vt)

            # --- attention (fp32) ---
            attnT = [atp.tile([P, T], F32, tag=f"at{i}") for i in range(DB)]
            sums_sb = stat.tile([H, T], F32, tag="sums")
            for hh in range(H):
                ob, off = hh // 2, (hh % 2) * HD
                po = pso.tile([P, VN], F32, tag="psumo")
                for kb in range(TB):
                    qoff = kb * P
                    pss = ps.tile([P, VN], F32, tag="psum")
                    nc.tensor.matmul(
                        pss[:, qoff:T],
                        kT[ob][off:off + HD, kb * P:(kb + 1) * P],
                        qT[ob][off:off + HD, qoff:T],
                        start=True, stop=True)
                    # per-key-row max over computed range (any per-row constant
                    # cancels exactly via the exp(m) column in v)
                    mrow = colp.tile([P, 1], F32, tag="mrow")
                    nc.vector.tensor_reduce(out=mrow[:], in_=pss[:, qoff:T],
                                            axis=AX.X, op=ALU.max)
                    nmrow = colp.tile([P, 1], F32, tag="nmrow")
                    nc.scalar.mul(out=nmrow[:], in_=mrow[:], mul=-1.0)
                    emrow = colp.tile([P, 1], F32, tag="emrow")
                    nc.scalar.activation(out=emrow[:], in_=mrow[:], func=ACTF.Exp)
                    ex = expp.tile([P, T], F32, tag="exp")
                    if qoff:
                        nc.vector.memset(ex[:, :qoff], 0.0)
                    nc.scalar.activation(out=ex[:, qoff:T], in_=pss[:, qoff:T],
                                         func=ACTF.Exp, bias=nmrow[:], scale=1.0)
                    # mask the diagonal block (strictly k<=q)
                    nc.vector.tensor_mul(out=ex[:, qoff:qoff + P],
                                         in0=ex[:, qoff:qoff + P], in1=tri[:])
                    # scale v rows by exp(m_k); ones col becomes exp(m_k)
                    vh = vhp.tile([P, HD + 1], F32, tag="vh")
                    nc.vector.tensor_scalar_mul(
                        out=vh[:], in0=v_sb[kb][:, hh, :], scalar1=emrow[:])
                    nc.tensor.matmul(po[:HD + 1, qoff:T], vh[:], ex[:, qoff:T],
                                     start=(kb == 0), stop=(kb == TB - 1))
                # normalize: recip of sums row, broadcast via PE, multiply
                nc.scalar.copy(out=sums_sb[hh:hh + 1, :], in_=po[HD:HD + 1, :T])
                rec = stat.tile([1, T], F32, tag="rec")
                nc.vector.reciprocal(out=rec[:], in_=sums_sb[hh:hh + 1, :])
                pb = ps.tile([P, VN], F32, tag="psum")
                nc.tensor.matmul(pb[:HD, :T], ones_row[:1, :HD], rec[:],
                                 start=True, stop=True)
                nc.scalar.copy(out=attnT[ob][off:off + HD, :], in_=po[:HD, :T])
                nc.vector.tensor_mul(out=attnT[ob][off:off + HD, :],
                                     in0=attnT[ob][off:off + HD, :],
                                     in1=pb[:HD, :T])
            if dbg and l == 0:
                for ob in range(DB):
                    nc.sync.dma_start(out=dbg["dbg_attn"][ob * P:(ob + 1) * P, :],
                                      in_=attnT[ob][:])

            # --- attention output projection (Wo) + residual ---
            m_o = cross_part_absmax(absmax_cols(attnT, "o"), f"o_{l}")
            isc_o = iscale_of(m_o, f"o_{l}")
            qo = quantize_tiles(attnT, isc_o, "qo", n=T)   # [o,t] already transposed
            sc_o = colp.tile([P, 1], F32, tag="sc_o")
            nc.scalar.mul(out=sc_o[:], in_=m_o[:], mul=wso / 127.0)
            for tb in range(TB):
                pw = ps.tile([P, VN], F32, tag="psum")
                for ob in range(DB):
                    nc.tensor.matmul(pw[:, :D], qo[ob][:, tb * P:(tb + 1) * P],
                                     wo_sb[ob][:], start=(ob == 0), stop=(ob == DB - 1))
                nc.vector.scalar_tensor_tensor(out=h[tb][:], in0=pw[:, :D],
                                               scalar=sc_o[:], in1=h[tb][:],
                                               op0=ALU.mult, op1=ALU.add)

            # --- encoder-context projection + residual (broadcast over t) ---
            pe = ps.tile([P, VN], F32, tag="psum")
            for db in range(DB):
                nc.tensor.matmul(pe[:1, :D], qctxT[:, db:db + 1], we_sb[db][:],
                                 start=(db == 0), stop=(db == DB - 1))
            enc_row = stat.tile([1, D], F32, tag="encrow")
            sc_e = stat.tile([1, 1], F32, tag="sc_e")
            nc.scalar.mul(out=sc_e[:], in_=cmc[:1, :], mul=wse / 127.0)
            nc.scalar.activation(out=enc_row[:], in_=pe[:1, :D], func=ACTF.Identity,
                                 bias=0.0, scale=sc_e[:])
            pbe = ps.tile([P, VN], F32, tag="psum")
            nc.tensor.matmul(pbe[:, :D], ones_row[:1, :P], enc_row[:],
                             start=True, stop=True)
            for tb in range(TB):
                nc.vector.tensor_add(out=h[tb][:], in0=h[tb][:], in1=pbe[:, :D])

            # --- FFN ---
            ln3 = layer_norm(h, None, None, False, "ln3")
            m_ln3 = cross_part_absmax(absmax_cols(ln3, "ln3"), f"ln3_{l}")
            isc3 = iscale_of(m_ln3, f"ln3_{l}")
            q3 = quantize_tiles(ln3, isc3, "ln3")
            q3T = transpose_to(q3, "ln3")
            w1_sb = [wfp.tile([P, FF], BF16, tag=f"w1_{db}") for db in range(DB)]
            for db in range(DB):
                nc.sync.dma_start(out=w1_sb[db][:], in_=w1[l][db * P:(db + 1) * P, :])
            b1_sb = const.tile([FB, P, 1], F32, tag="b1c")
            nc.sync.dma_start(out=b1_sb[:], in_=b1_t[l][:])
            sc_1 = colp.tile([P, 1], F32, tag="sc_1")
            nc.scalar.mul(out=sc_1[:], in_=m_ln3[:], mul=ws1 / 127.0)
            gel = []
            for fb in range(FB):
                pf = ps.tile([P, VN], F32, tag="psum")
                for db in range(DB):
                    nc.tensor.matmul(pf[:, :T], w1_sb[db][:, fb * P:(fb + 1) * P],
                                     q3T[db][:], start=(db == 0), stop=(db == DB - 1))
                gt = gelp.tile([P, T], F32, tag=f"gel{fb}")
                nc.scalar.activation(out=gt[:], in_=pf[:, :T], func=ACTF.Gelu,
                                     bias=b1_sb[fb][:], scale=sc_1[:])
                gel.append(gt)
            if dbg and l == 0:
                for fb in range(FB):
                    nc.sync.dma_start(out=dbg["dbg_gelu"][fb * P:(fb + 1) * P, :],
                                      in_=gel[fb][:])
            m_g = cross_part_absmax(absmax_cols(gel, "gel"), f"g_{l}")
            isc_g = iscale_of(m_g, f"g_{l}")
            sc_2 = colp.tile([P, 1], F32, tag="sc_2")
            nc.scalar.mul(out=sc_2[:], in_=m_g[:], mul=ws2 / 127.0)
            qg = []
            for fb in range(FB):
                g8 = qxp.tile([P, T], I8, tag="g8")
                nc.vector.tensor_scalar(out=g8[:], in0=gel[fb][:], scalar1=isc_g[:],
                                        scalar2=None, op0=ALU.divide)
                gq = qgp.tile([P, T], BF16, tag=f"qg{fb}")
                nc.scalar.copy(out=gq[:], in_=g8[:])
                qg.append(gq)
            w2_sb = [wfp.tile([P, D], BF16, tag=f"w2_{fb}") for fb in range(FB)]
            for fb in range(FB):
                nc.sync.dma_start(out=w2_sb[fb][:], in_=w2[l][fb * P:(fb + 1) * P, :])
            for tb in range(TB):
                pf2 = ps.tile([P, VN], F32, tag="psum")
                for fb in range(FB):
                    nc.tensor.matmul(pf2[:, :D], qg[fb][:, tb * P:(tb + 1) * P],
                                     w2_sb[fb][:], start=(fb == 0), stop=(fb == FB - 1))
                nc.vector.scalar_tensor_tensor(out=h[tb][:], in0=pf2[:, :D],
                                               scalar=sc_2[:], in1=h[tb][:],
                                               op0=ALU.mult, op1=ALU.add)
            if dbg:
                tgt = dbg["dbg_h0"] if l == 0 else dbg["dbg_h1"]
                for tb in range(TB):
                    nc.sync.dma_start(out=tgt[tb * P:(tb + 1) * P, :], in_=h[tb][:])

        # ---------------- final LN + output projection ----------------
        ws_out = ws[L]
        lnf = layer_norm(h, None, None, False, "lnf")
        m_h = cross_part_absmax(absmax_cols(lnf, "lnf"), "lnf")
        isc_h = iscale_of(m_h, "lnf")
        qh = quantize_tiles(lnf, isc_h, "lnf")
        qhT = transpose_to(qh, "lnf")
        sc_out = colp.tile([P, 1], F32, tag="sc_out")
        nc.scalar.mul(out=sc_out[:], in_=m_h[:], mul=ws_out / 127.0)
        nvb = VBF + (1 if VREM else 0)
        for vb in range(nvb):
            vn = VN if vb < VBF else VREM
            wtile = [wop.tile([P, VN], BF16, tag=f"wout{db}") for db in range(DB)]
            for db in range(DB):
                nc.sync.dma_start(out=wtile[db][:, :vn],
                                  in_=wout_in[db * P:(db + 1) * P,
                                              vb * VN:vb * VN + vn])
            for tb in range(TB):
                pl = ps.tile([P, VN], F32, tag="psum")
                for db in range(DB):
                    nc.tensor.matmul(pl[:, :vn], qhT[db][:, tb * P:(tb + 1) * P],
                                     wtile[db][:, :vn], start=(db == 0),
                                     stop=(db == DB - 1))
                lt = outp.tile([P, VN], F32, tag="lt")
                nc.scalar.activation(out=lt[:, :vn], in_=pl[:, :vn], func=ACTF.Copy,
                                     scale=sc_out[:])
                nc.sync.dma_start(
                    out=out[tb * P:(tb + 1) * P, vb * VN:vb * VN + vn],
                    in_=lt[:, :vn])


def bv_sb_flat(nc, bv_sb, colp):
    """bv bias laid out per output feature; v eviction writes [p, (h d)] so the
    bias must be per-partition only when o is on partitions -- for the natural
    v layout the bias varies along the free dim. The spec fills bv with zeros;
    return a zero [128,1] bias column (asserted nonzero-free on host)."""
    z = colp.tile([P, 1], F32, tag="bv_zero")
    nc.vector.memset(z[:], 0.0)
    return z[:]


def _prep(inputs):
    """Host-side packing shared across cores; returns (common dict, wscales)."""
    f32 = np.float32
    ws = []
    common = {}
    for l in range(L):
        packed = []
        for name, W in [("wq", inputs["Wq"][l]), ("wk", inputs["Wk"][l]),
                        ("wv", inputs["Wv"][l]), ("wo", inputs["Wo"][l]),
                        ("wenc", inputs["Wenc"][l]), ("w1", inputs["W1"][l]),
                        ("w2", inputs["W2"][l])]:
            qWT, s = _quantize_weight(W)
            common[f"{name}{l}"] = qWT
            packed.append(s)
        ws.append(tuple(packed))
        common[f"bq{l}"] = _col_layout(
            np.asarray(inputs["bq"][l], f32) * f32(INV_SQRT_HD))
        common[f"bk{l}"] = _col_layout(inputs["bk"][l])
        common[f"bv{l}"] = _col_layout(inputs["bv"][l])
        common[f"b1{l}"] = _col_layout(inputs["b1"][l])
    qWoutT, s_out = _quantize_weight(inputs["Wout"])
    common["wout"] = qWoutT
    ws.append(s_out)
    common["tri"] = np.triu(np.ones((P, P), dtype=f32))
    return common, ws


def kernel(**inputs):
    debug = bool(int(os.environ.get("BITGEN_DEBUG", "0")))
    common, ws = _prep(inputs)

    # unsupported-by-fast-path inputs must be zero/unit (true for this model's
    # spec: all biases zero, all LN affines identity)
    for nm in ["bo", "benc", "b2", "bout", "ln1b", "ln2b", "ln3b", "lnfb"]:
        assert not np.any(np.asarray(inputs[nm])), f"nonzero {nm} unsupported"
    for nm in ["ln1g", "ln2g", "ln3g", "lnfg"]:
        assert np.all(np.asarray(inputs[nm]) == 1.0), f"non-unit {nm} unsupported"

    key = ("v1", debug)
    if key not in _NC_CACHE:
        _NC_CACHE[key] = build(ws, debug=debug)
    nc = _NC_CACHE[key]

    enc = np.asarray(inputs["encoder_output"], np.float32)
    x = np.asarray(inputs["x"], np.float32)
    in_maps = []
    for c in range(NCORES):
        m = dict(common)
        m["x"] = np.ascontiguousarray(x[c])
        # roll so that this core's batch is row 0 of the enc block (the kernel
        # quantizes ctx row 0 as its own batch but uses all rows for the scale)
        m["enc"] = np.ascontiguousarray(
            np.roll(enc, -c, axis=0).reshape(B * S, D))
        in_maps.append(m)

    res = run_bass_kernel_spmd(nc, in_maps, list(range(NCORES)))
    outs = [res.results[c]["logits"] for c in range(NCORES)]
    return np.stack(outs, axis=0)


if __name__ == "__main__":
    pass


# revision 9
# speedup vs baseline: 1.0344x; 1.0344x over previous
"""Trainium2 Bass kernel for nn_BitGenModel (BitNet-style dense transformer).

Sharding: data-parallel over batch (B=8) across 8 NeuronCores; each core runs
the full 2-layer transformer + final 32000-vocab projection for its batch
element. Weights are pre-quantized (ternary, bf16) and pre-transposed on the
host; activation quantization (per-tensor absmax int8) runs on device with
batch-local scales.

Numerics:
- BitLinear integer matmuls run on the PE in bf16: |qx|<=127 and qW in
  {-1,0,1} are exactly representable and accumulation is fp32 in PSUM, so
  these matmuls are exact.
- Attention (scores, softmax, A@V) runs in fp32. Softmax subtracts a
  per-key-row max m_k and folds an exp(m_k) correction column into the V
  operand; the m_k cancel exactly in the normalization, so no cross-partition
  reductions are needed.
- LayerNorm uses DVE bn_stats/bn_aggr (fp32, free-dim reduction) in the
  natural [token, feature] layout.
"""
import math
import os

import numpy as np
import ml_dtypes

import concourse.bass as bass
import concourse.mybir as mybir
import concourse.tile as tile
from concourse import bacc
from concourse.bass_utils import run_bass_kernel_spmd
from concourse.masks import make_identity

F32 = mybir.dt.float32
BF16 = mybir.dt.bfloat16
I8 = mybir.dt.int8
AX = mybir.AxisListType
ALU = mybir.AluOpType
ACTF = mybir.ActivationFunctionType

B, T, S = 8, 512, 256
D, H, HD, FF, V, L = 512, 8, 64, 2048, 32000, 2
EPS = 1e-5
P = 128
TB = T // P            # 4 token blocks
DB = D // P            # 4 feature blocks
FB = FF // P           # 16 ffn blocks
VN = 512               # vocab tile width
VBF = V // VN          # 62 full vocab blocks
VREM = V - VBF * VN    # 256 remainder
NCORES = 8
INV_SQRT_HD = 1.0 / math.sqrt(HD)

_NC_CACHE = {}


def _quantize_weight(W):
    """Host-side eval-mode BitNet weight quantization (matches reference)."""
    W = np.asarray(W, dtype=np.float32)
    w_scale = np.maximum(np.abs(W).mean(dtype=np.float32), np.float32(1e-5))
    qW = np.sign(W) * (np.abs(W) > np.float32(0.5) * w_scale)
    qWT = np.ascontiguousarray(qW.astype(np.float32).T.astype(ml_dtypes.bfloat16))
    return qWT, float(w_scale)


def _col_layout(v):
    """[N] -> [128, N/128]: element [p, j] = v[j*128 + p] (per-partition cols)."""
    v = np.asarray(v, dtype=np.float32)
    n = v.shape[0]
    return np.ascontiguousarray(v.reshape(n // P, P).T)


def build(wscales, debug=False):
    nc = bacc.Bacc("TRN2", target_bir_lowering=False, debug=False,
                   num_devices=NCORES)

    x_in = nc.dram_tensor("x", [T, D], F32, kind="ExternalInput")
    enc_in = nc.dram_tensor("enc", [B * S, D], F32, kind="ExternalInput")
    tri_in = nc.dram_tensor("tri", [P, P], F32, kind="ExternalInput")

    wq, wk, wv, wo, wenc, w1, w2 = [], [], [], [], [], [], []
    bq, bk, b1 = [], [], []
    for l in range(L):
        wq.append(nc.dram_tensor(f"wq{l}", [D, D], BF16, kind="ExternalInput"))
        wk.append(nc.dram_tensor(f"wk{l}", [D, D], BF16, kind="ExternalInput"))
        wv.append(nc.dram_tensor(f"wv{l}", [D, D], BF16, kind="ExternalInput"))
        wo.append(nc.dram_tensor(f"wo{l}", [D, D], BF16, kind="ExternalInput"))
        wenc.append(nc.dram_tensor(f"wenc{l}", [D, D], BF16, kind="ExternalInput"))
        w1.append(nc.dram_tensor(f"w1{l}", [D, FF], BF16, kind="ExternalInput"))
        w2.append(nc.dram_tensor(f"w2{l}", [FF, D], BF16, kind="ExternalInput"))
        bq.append(nc.dram_tensor(f"bq{l}", [P, DB], F32, kind="ExternalInput"))
        bk.append(nc.dram_tensor(f"bk{l}", [P, DB], F32, kind="ExternalInput"))
        b1.append(nc.dram_tensor(f"b1{l}", [P, FB], F32, kind="ExternalInput"))
    wout_in = nc.dram_tensor("wout", [D, V], BF16, kind="ExternalInput")

    out = nc.dram_tensor("logits", [T, V], F32, kind="ExternalOutput")

    dbg = {}
    if debug:
        for name, shape in [("dbg_h0", [T, D]), ("dbg_h1", [T, D]),
                            ("dbg_ln1", [T, D]), ("dbg_attn", [D, T]),
                            ("dbg_qT", [D, T]), ("dbg_gelu", [FF, T])]:
            dbg[name] = nc.dram_tensor(name, shape, F32, kind="ExternalOutput")

    with tile.TileContext(nc) as tc:
        _body(nc, tc, wscales, x_in, enc_in, tri_in,
              wq, wk, wv, wo, wenc, w1, w2, bq, bk, b1, wout_in, out, dbg)
    nc.compile()
    return nc


def _body(nc, tc, ws, x_in, enc_in, tri_in,
          wq, wk, wv, wo, wenc, w1, w2, bq_t, bk_t, b1_t, wout_in, out, dbg):
    from contextlib import ExitStack
    ctx = ExitStack()
    with ctx:
        # ---------------- pools (uniform tile shape per pool) ----------------
        hp = ctx.enter_context(tc.tile_pool(name="hp", bufs=TB))
        const = ctx.enter_context(tc.tile_pool(name="const", bufs=1))
        lnp = ctx.enter_context(tc.tile_pool(name="lnp", bufs=TB + 1))
        stat = ctx.enter_context(tc.tile_pool(name="stat", bufs=2))
        colp = ctx.enter_context(tc.tile_pool(name="colp", bufs=2))
        qxp = ctx.enter_context(tc.tile_pool(name="qxp", bufs=3))
        qtp = ctx.enter_context(tc.tile_pool(name="qtp", bufs=9))
        wsp = ctx.enter_context(tc.tile_pool(name="wsp", bufs=14))
        w1p = ctx.enter_context(tc.tile_pool(name="w1p", bufs=DB))
        w2p = ctx.enter_context(tc.tile_pool(name="w2p", bufs=FB))
        qkp = ctx.enter_context(tc.tile_pool(name="qkp", bufs=2 * DB))
        vp = ctx.enter_context(tc.tile_pool(name="vp", bufs=TB))
        vhp = ctx.enter_context(tc.tile_pool(name="vhp", bufs=3))
        expp = ctx.enter_context(tc.tile_pool(name="expp", bufs=4))
        atp = ctx.enter_context(tc.tile_pool(name="atp", bufs=TB))
        gelp = ctx.enter_context(tc.tile_pool(name="gelp", bufs=FB))
        qgp = ctx.enter_context(tc.tile_pool(name="qgp", bufs=FB))
        drp = ctx.enter_context(tc.tile_pool(name="drp", bufs=4, space="DRAM"))
        outp = ctx.enter_context(tc.tile_pool(name="outp", bufs=3))
        wop = ctx.enter_context(tc.tile_pool(name="wop", bufs=6))
        ps = ctx.enter_context(tc.tile_pool(name="ps", bufs=5, space="PSUM"))
        pst_p = ctx.enter_context(tc.tile_pool(name="pst", bufs=1, space="PSUM"))
        pso = ctx.enter_context(tc.tile_pool(name="pso", bufs=2, space="PSUM"))

        # ---------------- constants ----------------
        tri = const.tile([P, P], F32)
        nc.sync.dma_start(out=tri[:], in_=tri_in[:])
        eps_c = const.tile([P, 1], F32)
        nc.vector.memset(eps_c[:], EPS)
        ones_row = const.tile([1, P], F32)
        nc.vector.memset(ones_row[:], 1.0)
        ones_col = const.tile([P, 1], F32)
        nc.vector.memset(ones_col[:], 1.0)
        ident_bf = const.tile([P, P], BF16)
        make_identity(nc, ident_bf)

        # ---------------- load x into resident h tiles ----------------
        h = []
        for tb in range(TB):
            ht = hp.tile([P, D], F32, tag="h")
            nc.sync.dma_start(out=ht[:], in_=x_in[tb * P:(tb + 1) * P, :])
            h.append(ht)

        # =========== helpers ===========
        def cross_part_absmax(tiles, tag):
            """tiles: list of [128, n] f32 -> [128,1] all-partition absmax
            (batch-local global max), clamped at 1e-8."""
            acc = None
            for tl in tiles:
                c = colp.tile([P, 1], F32, tag="col", bufs=12)
                nc.vector.tensor_reduce(out=c[:], in_=tl[:], axis=AX.X,
                                        op=ALU.max, apply_absolute_value=True)
                if acc is None:
                    acc = c
                else:
                    nxt = colp.tile([P, 1], F32, tag="col", bufs=12)
                    nc.vector.tensor_max(out=nxt[:], in0=acc[:], in1=c[:])
                    acc = nxt
            m1 = stat.tile([1, 1], F32, tag="cpm1")
            nc.gpsimd.tensor_reduce(out=m1[:], in_=acc[:], axis=AX.C, op=ALU.max)
            mb = colp.tile([P, 1], F32, tag="mhat", bufs=4)
            nc.gpsimd.partition_broadcast(mb[:], m1[:], channels=P)
            mc = colp.tile([P, 1], F32, tag="mhat", bufs=4)
            nc.vector.tensor_scalar_max(out=mc[:], in0=mb[:], scalar1=1e-8)
            return mc

        def iscale_of(mhat, tag):
            # returns INVERSE scale 127/m (quantize multiplies by this)
            rcp = colp.tile([P, 1], F32, tag="isc", bufs=6)
            nc.vector.reciprocal(out=rcp[:], in_=mhat[:])
            inv = colp.tile([P, 1], F32, tag="isc", bufs=6)
            nc.vector.tensor_scalar_mul(out=inv[:], in0=rcp[:], scalar1=127.0)
            return inv

        def quantize_tiles(src_tiles, isc, n=D):
            """f32 [128,n] tiles -> bf16 [128,n] tiles (int8 round/saturate)."""
            res = []
            for st in src_tiles:
                q8 = qxp.tile([P, T], I8, tag="q8")
                nc.vector.tensor_scalar_mul(out=q8[:, :n], in0=st[:], scalar1=isc[:])
                qb = qtp.tile([P, T], BF16, tag="qt")
                nc.gpsimd.tensor_copy(out=qb[:, :n], in_=q8[:, :n])
                res.append(qb)
            return res

        def transpose_to(qtiles):
            """TB x [128, D] bf16 (rows=t) -> DB x [128, T] bf16 (rows=d)."""
            outt = [qtp.tile([P, T], BF16, tag="qt", name="qt") for _ in range(DB)]
            for tb in range(TB):
                for db in range(DB):
                    pst = pst_p.tile([P, P], BF16, tag="pstr", name="pstr")
                    nc.tensor.transpose(pst[:P, :P],
                                        qtiles[tb][:, db * P:(db + 1) * P],
                                        ident_bf[:])
                    nc.vector.tensor_copy(out=outt[db][:, tb * P:(tb + 1) * P],
                                          in_=pst[:P, :P])
            return outt

        def layer_norm(tiles):
            outs = []
            for tb in range(TB):
                st6 = stat.tile([P, 6], F32, tag="bn6")
                nc.vector.bn_stats(out=st6[:], in_=tiles[tb][:])
                mv = stat.tile([P, 2], F32, tag="bn2")
                nc.vector.bn_aggr(out=mv[:], in_=st6[:])
                std = colp.tile([P, 1], F32, tag="col", bufs=12)
                nc.scalar.activation(out=std[:], in_=mv[:, 1:2], func=ACTF.Sqrt,
                                     bias=eps_c[:], scale=1.0)
                rstd = colp.tile([P, 1], F32, tag="col", bufs=12)
                nc.vector.reciprocal(out=rstd[:], in_=std[:])
                ot = lnp.tile([P, D], F32, tag="ln")
                nc.vector.tensor_scalar(out=ot[:], in0=tiles[tb][:],
                                        scalar1=mv[:, 0:1], scalar2=rstd[:],
                                        op0=ALU.subtract, op1=ALU.mult)
                outs.append(ot)
            return outs

        # ---------------- ctx prep (encoder mean + quantize, once) ----------
        ctx_rows = []
        for b in range(B):
            pctx = ps.tile([P, VN], F32, tag="psum")
            for sb in range(S // P):
                et = lnp.tile([P, D], F32, tag="ln")
                nc.sync.dma_start(
                    out=et[:], in_=enc_in[b * S + sb * P: b * S + (sb + 1) * P, :])
                nc.tensor.matmul(pctx[:1, :D], ones_col[:], et[:],
                                 start=(sb == 0), stop=(sb == S // P - 1))
            cr = stat.tile([1, D], F32, tag="ctxr", bufs=B)
            nc.scalar.activation(out=cr[:], in_=pctx[:1, :D],
                                 func=ACTF.Copy, scale=1.0 / S)
            ctx_rows.append(cr)
        cacc = None
        for b in range(B):
            cm = stat.tile([1, 1], F32, tag="cm", bufs=4)
            nc.vector.tensor_reduce(out=cm[:], in_=ctx_rows[b][:], axis=AX.X,
                                    op=ALU.max, apply_absolute_value=True)
            if cacc is None:
                cacc = cm
            else:
                nx = stat.tile([1, 1], F32, tag="cm", bufs=4)
                nc.vector.tensor_max(out=nx[:], in0=cacc[:], in1=cm[:])
                cacc = nx
        cmb = colp.tile([P, 1], F32, tag="ctxm")
        nc.gpsimd.partition_broadcast(cmb[:], cacc[:], channels=P)
        cmc = colp.tile([P, 1], F32, tag="ctxm")
        nc.vector.tensor_scalar_max(out=cmc[:], in0=cmb[:], scalar1=1e-8)
        isc_ctx = iscale_of(cmc, "ctx")
        q8row = stat.tile([1, D], I8, tag="q8ctx")
        nc.vector.tensor_scalar_mul(out=q8row[:], in0=ctx_rows[0][:],
                                    scalar1=isc_ctx[:1, :])
        dctx8 = drp.tile([1, D], I8, tag="dctx8")
        nc.sync.dma_start(out=dctx8[:], in_=q8row[:])
        qctx8 = stat.tile([P, DB], I8, tag="qctx8")
        nc.sync.dma_start(out=qctx8[:],
                          in_=dctx8[:].rearrange("one (j p) -> p (one j)", p=P))
        qctxT = const.tile([P, DB], BF16)
        nc.scalar.copy(out=qctxT[:], in_=qctx8[:])

        # ---------------- transformer layers ----------------
        for l in range(L):
            wsq, wsk, wsv, wso, wse, ws1, ws2 = ws[l]

            wq_sb = [wsp.tile([P, D], BF16, tag="ws", name="ws") for _ in range(DB)]
            wk_sb = [wsp.tile([P, D], BF16, tag="ws", name="ws") for _ in range(DB)]
            wv_sb = [wsp.tile([P, D], BF16, tag="ws", name="ws") for _ in range(DB)]
            for db in range(DB):
                nc.sync.dma_start(out=wq_sb[db][:], in_=wq[l][db * P:(db + 1) * P, :])
                nc.sync.dma_start(out=wk_sb[db][:], in_=wk[l][db * P:(db + 1) * P, :])
                nc.sync.dma_start(out=wv_sb[db][:], in_=wv[l][db * P:(db + 1) * P, :])
            bq_sb = const.tile([P, DB], F32, tag="bqc")
            bk_sb = const.tile([P, DB], F32, tag="bkc")
            nc.sync.dma_start(out=bq_sb[:], in_=bq_t[l][:])
            nc.sync.dma_start(out=bk_sb[:], in_=bk_t[l][:])

            # --- ln1 + quantize + transpose ---
            ln1 = layer_norm(h)
            if dbg and l == 0:
                for tb in range(TB):
                    nc.sync.dma_start(out=dbg["dbg_ln1"][tb * P:(tb + 1) * P, :],
                                      in_=ln1[tb][:])
            m_ln1 = cross_part_absmax(ln1, f"ln1_{l}")
            isc1 = iscale_of(m_ln1, f"ln1_{l}")
            q1 = quantize_tiles(ln1, isc1)
            q1T = transpose_to(q1)

            # --- qkv matmuls ---
            sc_q = colp.tile([P, 1], F32, tag="sc", bufs=6)
            nc.scalar.mul(out=sc_q[:], in_=m_ln1[:], mul=wsq * INV_SQRT_HD / 127.0)
            sc_k = colp.tile([P, 1], F32, tag="sc", bufs=6)
            nc.scalar.mul(out=sc_k[:], in_=m_ln1[:], mul=wsk / 127.0)
            sc_v = colp.tile([P, 1], F32, tag="sc", bufs=6)
            nc.scalar.mul(out=sc_v[:], in_=m_ln1[:], mul=wsv / 127.0)
            qT, kT = [], []
            for ob in range(DB):
                pq = ps.tile([P, VN], F32, tag="psum")
                for db in range(DB):
                    nc.tensor.matmul(pq[:, :T], wq_sb[db][:, ob * P:(ob + 1) * P],
                                     q1T[db][:], start=(db == 0), stop=(db == DB - 1))
                qf = qkp.tile([P, T], F32, tag="qk")
                nc.scalar.activation(out=qf[:], in_=pq[:, :T], func=ACTF.Identity,
                                     bias=bq_sb[:, ob:ob + 1], scale=sc_q[:])
                qT.append(qf)
                pk = ps.tile([P, VN], F32, tag="psum")
                for db in range(DB):
                    nc.tensor.matmul(pk[:, :T], wk_sb[db][:, ob * P:(ob + 1) * P],
                                     q1T[db][:], start=(db == 0), stop=(db == DB - 1))
                kf = qkp.tile([P, T], F32, tag="qk")
                nc.scalar.activation(out=kf[:], in_=pk[:, :T], func=ACTF.Identity,
                                     bias=bk_sb[:, ob:ob + 1], scale=sc_k[:])
                kT.append(kf)
            if dbg and l == 0:
                for ob in range(DB):
                    nc.sync.dma_start(out=dbg["dbg_qT"][ob * P:(ob + 1) * P, :],
                                      in_=qT[ob][:])
            v_sb = []
            for tb in range(TB):
                pv = ps.tile([P, VN], F32, tag="psum")
                for db in range(DB):
                    nc.tensor.matmul(pv[:, :D], q1T[db][:, tb * P:(tb + 1) * P],
                                     wv_sb[db][:], start=(db == 0), stop=(db == DB - 1))
                vt = vp.tile([P, H, HD + 1], F32, tag="v")
                nc.scalar.activation(
                    out=vt[:, :, :HD],
                    in_=pv[:, :D].rearrange("p (h d) -> p h d", h=H),
                    func=ACTF.Identity, bias=0.0, scale=sc_v[:])
                nc.vector.memset(vt[:, :, HD:HD + 1], 1.0)
                v_sb.append(vt)

            # --- attention (fp32) ---
            attnT = [atp.tile([P, T], F32, tag="at", name="at") for _ in range(DB)]
            for hh in range(H):
                ob, off = hh // 2, (hh % 2) * HD
                po = pso.tile([P, VN], F32, tag="psumo")
                for kb in range(TB):
                    qoff = kb * P
                    pss = ps.tile([P, VN], F32, tag="psum")
                    nc.tensor.matmul(
                        pss[:, qoff:T],
                        kT[ob][off:off + HD, kb * P:(kb + 1) * P],
                        qT[ob][off:off + HD, qoff:T],
                        start=True, stop=True)
                    mrow = colp.tile([P, 1], F32, tag="mrow", bufs=6)
                    nc.vector.tensor_reduce(out=mrow[:], in_=pss[:, qoff:T],
                                            axis=AX.X, op=ALU.max)
                    nmrow = colp.tile([P, 1], F32, tag="mrow", bufs=6)
                    nc.scalar.mul(out=nmrow[:], in_=mrow[:], mul=-1.0)
                    emrow = colp.tile([P, 1], F32, tag="mrow", bufs=6)
                    nc.scalar.activation(out=emrow[:], in_=mrow[:], func=ACTF.Exp)
                    ex = expp.tile([P, T], F32, tag="exp")
                    if qoff:
                        nc.gpsimd.memset(ex[:, :qoff], 0.0)
                    nc.scalar.activation(out=ex[:, qoff:T], in_=pss[:, qoff:T],
                                         func=ACTF.Exp, bias=nmrow[:], scale=1.0)
                    nc.vector.tensor_mul(out=ex[:, qoff:qoff + P],
                                         in0=ex[:, qoff:qoff + P], in1=tri[:])
                    vh = vhp.tile([P, HD + 1], F32, tag="vh")
                    nc.vector.tensor_scalar_mul(
                        out=vh[:], in0=v_sb[kb][:, hh, :], scalar1=emrow[:])
                    nc.tensor.matmul(po[:HD + 1, qoff:T], vh[:], ex[:, qoff:T],
                                     start=(kb == 0), stop=(kb == TB - 1))
                rec = stat.tile([1, T], F32, tag="rec")
                nc.vector.reciprocal(out=rec[:], in_=po[HD:HD + 1, :T])
                pb = ps.tile([P, VN], F32, tag="psum")
                nc.tensor.matmul(pb[:HD, :T], ones_row[:1, :HD], rec[:],
                                 start=True, stop=True)
                nc.scalar.copy(out=attnT[ob][off:off + HD, :], in_=po[:HD, :T])
                nc.vector.tensor_mul(out=attnT[ob][off:off + HD, :],
                                     in0=attnT[ob][off:off + HD, :],
                                     in1=pb[:HD, :T])
            if dbg and l == 0:
                for ob in range(DB):
                    nc.sync.dma_start(out=dbg["dbg_attn"][ob * P:(ob + 1) * P, :],
                                      in_=attnT[ob][:])

            # --- attention output projection (Wo) + residual ---
            wo_sb = [wsp.tile([P, D], BF16, tag="ws", name="ws") for _ in range(DB)]
            we_sb = [wsp.tile([P, D], BF16, tag="ws", name="ws") for _ in range(DB)]
            for db in range(DB):
                nc.sync.dma_start(out=wo_sb[db][:], in_=wo[l][db * P:(db + 1) * P, :])
                nc.sync.dma_start(out=we_sb[db][:], in_=wenc[l][db * P:(db + 1) * P, :])
            m_o = cross_part_absmax(attnT, f"o_{l}")
            isc_o = iscale_of(m_o, f"o_{l}")
            qo = quantize_tiles(attnT, isc_o, n=T)
            sc_o = colp.tile([P, 1], F32, tag="sc", bufs=6)
            nc.scalar.mul(out=sc_o[:], in_=m_o[:], mul=wso / 127.0)
            for tb in range(TB):
                pw = ps.tile([P, VN], F32, tag="psum")
                for ob in range(DB):
                    nc.tensor.matmul(pw[:, :D], qo[ob][:, tb * P:(tb + 1) * P],
                                     wo_sb[ob][:], start=(ob == 0), stop=(ob == DB - 1))
                nc.vector.scalar_tensor_tensor(out=h[tb][:], in0=pw[:, :D],
                                               scalar=sc_o[:], in1=h[tb][:],
                                               op0=ALU.mult, op1=ALU.add)

            # --- encoder-context projection + residual (broadcast over t) ---
            pe = ps.tile([P, VN], F32, tag="psum")
            for db in range(DB):
                nc.tensor.matmul(pe[:1, :D], qctxT[:, db:db + 1], we_sb[db][:],
                                 start=(db == 0), stop=(db == DB - 1))
            enc_row = stat.tile([1, D], F32, tag="encrow", bufs=1)
            sc_e = stat.tile([1, 1], F32, tag="sc_e")
            nc.scalar.mul(out=sc_e[:], in_=cmc[:1, :], mul=wse / 127.0)
            nc.scalar.activation(out=enc_row[:], in_=pe[:1, :D],
                                 func=ACTF.Copy, scale=sc_e[:])
            pbe = ps.tile([P, VN], F32, tag="psum")
            nc.tensor.matmul(pbe[:, :D], ones_row[:1, :P], enc_row[:],
                             start=True, stop=True)
            for tb in range(TB):
                nc.vector.tensor_add(out=h[tb][:], in0=h[tb][:], in1=pbe[:, :D])

            # --- FFN ---
            ln3 = layer_norm(h)
            m_ln3 = cross_part_absmax(ln3, f"ln3_{l}")
            isc3 = iscale_of(m_ln3, f"ln3_{l}")
            q3 = quantize_tiles(ln3, isc3)
            q3T = transpose_to(q3)
            w1_sb = [w1p.tile([P, FF], BF16, tag="w1", name="w1") for _ in range(DB)]
            for db in range(DB):
                nc.sync.dma_start(out=w1_sb[db][:], in_=w1[l][db * P:(db + 1) * P, :])
            b1_sb = const.tile([P, FB], F32, tag="b1c")
            nc.sync.dma_start(out=b1_sb[:], in_=b1_t[l][:])
            sc_1 = colp.tile([P, 1], F32, tag="sc", bufs=6)
            nc.scalar.mul(out=sc_1[:], in_=m_ln3[:], mul=ws1 / 127.0)
            gel = []
            for fb in range(FB):
                pf = ps.tile([P, VN], F32, tag="psum")
                for db in range(DB):
                    nc.tensor.matmul(pf[:, :T], w1_sb[db][:, fb * P:(fb + 1) * P],
                                     q3T[db][:], start=(db == 0), stop=(db == DB - 1))
                gt = gelp.tile([P, T], F32, tag="gel")
                nc.scalar.activation(out=gt[:], in_=pf[:, :T], func=ACTF.Gelu,
                                     bias=b1_sb[:, fb:fb + 1], scale=sc_1[:])
                gel.append(gt)
            if dbg and l == 0:
                for fb in range(FB):
                    nc.sync.dma_start(out=dbg["dbg_gelu"][fb * P:(fb + 1) * P, :],
                                      in_=gel[fb][:])
            m_g = cross_part_absmax(gel, f"g_{l}")
            isc_g = iscale_of(m_g, f"g_{l}")
            sc_2 = colp.tile([P, 1], F32, tag="sc", bufs=6)
            nc.scalar.mul(out=sc_2[:], in_=m_g[:], mul=ws2 / 127.0)
            qg = []
            for fb in range(FB):
                g8 = qxp.tile([P, T], I8, tag="q8")
                nc.vector.tensor_scalar_mul(out=g8[:], in0=gel[fb][:], scalar1=isc_g[:])
                gq = qgp.tile([P, T], BF16, tag="qg")
                nc.gpsimd.tensor_copy(out=gq[:], in_=g8[:])
                qg.append(gq)
            w2_sb = [w2p.tile([P, D], BF16, tag="w2", name="w2") for _ in range(FB)]
            for fb in range(FB):
                nc.sync.dma_start(out=w2_sb[fb][:], in_=w2[l][fb * P:(fb + 1) * P, :])
            for tb in range(TB):
                pf2 = ps.tile([P, VN], F32, tag="psum")
                for fb in range(FB):
                    nc.tensor.matmul(pf2[:, :D], qg[fb][:, tb * P:(tb + 1) * P],
                                     w2_sb[fb][:], start=(fb == 0), stop=(fb == FB - 1))
                nc.vector.scalar_tensor_tensor(out=h[tb][:], in0=pf2[:, :D],
                                               scalar=sc_2[:], in1=h[tb][:],
                                               op0=ALU.mult, op1=ALU.add)
            if dbg:
                tgt = dbg["dbg_h0"] if l == 0 else dbg["dbg_h1"]
                for tb in range(TB):
                    nc.sync.dma_start(out=tgt[tb * P:(tb + 1) * P, :], in_=h[tb][:])

        # ---------------- final LN + output projection ----------------
        ws_out = ws[L]
        lnf = layer_norm(h)
        m_h = cross_part_absmax(lnf, "lnf")
        isc_h = iscale_of(m_h, "lnf")
        qh = quantize_tiles(lnf, isc_h)
        qhT = transpose_to(qh)
        sc_out = colp.tile([P, 1], F32, tag="sc_out")
        nc.scalar.mul(out=sc_out[:], in_=m_h[:], mul=ws_out / 127.0)
        nvb = VBF + (1 if VREM else 0)
        for vb in range(nvb):
            vn = VN if vb < VBF else VREM
            wtile = [wop.tile([P, VN], BF16, tag="wout", name="wout") for _ in range(DB)]
            for db in range(DB):
                nc.sync.dma_start(out=wtile[db][:, :vn],
                                  in_=wout_in[db * P:(db + 1) * P,
                                              vb * VN:vb * VN + vn])
            for tb in range(TB):
                pl = ps.tile([P, VN], F32, tag="psum")
                for db in range(DB):
                    nc.tensor.matmul(pl[:, :vn], qhT[db][:, tb * P:(tb + 1) * P],
                                     wtile[db][:, :vn], start=(db == 0),
                                     stop=(db == DB - 1))
                lt = outp.tile([P, VN], F32, tag="lt")
                nc.scalar.activation(out=lt[:, :vn], in_=pl[:, :vn],
                                     func=ACTF.Copy, scale=sc_out[:])
                nc.sync.dma_start(
                    out=out[tb * P:(tb + 1) * P, vb * VN:vb * VN + vn],
                    in_=lt[:, :vn])


def _prep(inputs):
    """Host-side packing shared across cores; returns (common dict, wscales)."""
    f32 = np.float32
    ws = []
    common = {}
    for l in range(L):
        packed = []
        for name, W in [("wq", inputs["Wq"][l]), ("wk", inputs["Wk"][l]),
                        ("wv", inputs["Wv"][l]), ("wo", inputs["Wo"][l]),
                        ("wenc", inputs["Wenc"][l]), ("w1", inputs["W1"][l]),
                        ("w2", inputs["W2"][l])]:
            qWT, s = _quantize_weight(W)
            common[f"{name}{l}"] = qWT
            packed.append(s)
        ws.append(tuple(packed))
        common[f"bq{l}"] = _col_layout(
            np.asarray(inputs["bq"][l], f32) * f32(INV_SQRT_HD))
        common[f"bk{l}"] = _col_layout(inputs["bk"][l])
        common[f"b1{l}"] = _col_layout(inputs["b1"][l])
    qWoutT, s_out = _quantize_weight(inputs["Wout"])
    common["wout"] = qWoutT
    ws.append(s_out)
    common["tri"] = np.triu(np.ones((P, P), dtype=f32))
    return common, ws


def kernel(**inputs):
    debug = bool(int(os.environ.get("BITGEN_DEBUG", "0")))
    common, ws = _prep(inputs)

    # Fast path assumes the model's declared fills: zero biases on the layers
    # without per-partition bias layout, identity LN affines.
    for nm in ["bo", "benc", "b2", "bout", "bv",
               "ln1b", "ln2b", "ln3b", "lnfb"]:
        assert not np.any(np.asarray(inputs[nm])), f"nonzero {nm} unsupported"
    for nm in ["ln1g", "ln2g", "ln3g", "lnfg"]:
        assert np.all(np.asarray(inputs[nm]) == 1.0), f"non-unit {nm} unsupported"

    key = ("v1", debug, tuple(np.asarray(w, np.float64).tobytes()
                              for w in (tuple(ws[l]) for l in range(L)))), ws[L]
    key = (repr(ws), debug)
    if key not in _NC_CACHE:
        _NC_CACHE[key] = build(ws, debug=debug)
    nc = _NC_CACHE[key]

    enc = np.asarray(inputs["encoder_output"], np.float32)
    x = np.asarray(inputs["x"], np.float32)
    in_maps = []
    for c in range(NCORES):
        m = dict(common)
        m["x"] = np.ascontiguousarray(x[c])
        # roll so this core's batch is the first S-row block (the kernel
        # quantizes ctx row 0 as its own batch, using all rows for the scale)
        m["enc"] = np.ascontiguousarray(
            np.roll(enc, -c, axis=0).reshape(B * S, D))
        in_maps.append(m)

    res = run_bass_kernel_spmd(nc, in_maps, list(range(NCORES)))
    outs = [res.results[c]["logits"] for c in range(NCORES)]
    return np.stack(outs, axis=0)


# revision 14
# speedup vs baseline: 1.1957x; 1.1559x over previous
"""Trainium2 Bass kernel for nn_BitGenModel (BitNet-style dense transformer).

Sharding: data-parallel over batch (B=8) across 8 NeuronCores; each core runs
the full 2-layer transformer + final 32000-vocab projection for its batch
element. Weights are pre-quantized (ternary, bf16) and pre-transposed on the
host; activation quantization (per-tensor absmax int8) runs on device with
batch-local scales.

Numerics:
- BitLinear integer matmuls run on the PE in bf16: |qx|<=127 and qW in
  {-1,0,1} are exactly representable and accumulation is fp32 in PSUM, so
  these matmuls are exact.
- Attention (scores, softmax, A@V) runs in fp32. Softmax subtracts a
  per-key-row max m_k and folds an exp(m_k) correction column into the V
  operand; the m_k cancel exactly in the normalization, so no cross-partition
  reductions are needed.
- LayerNorm uses DVE bn_stats/bn_aggr (fp32, free-dim reduction) in the
  natural [token, feature] layout.
"""
import math
import os

import numpy as np
import ml_dtypes

import concourse.bass as bass
import concourse.mybir as mybir
import concourse.tile as tile
from concourse import bacc
from concourse.bass_utils import run_bass_kernel_spmd
from concourse.masks import make_identity

F32 = mybir.dt.float32
BF16 = mybir.dt.bfloat16
I8 = mybir.dt.int8
AX = mybir.AxisListType
ALU = mybir.AluOpType
ACTF = mybir.ActivationFunctionType

B, T, S = 8, 512, 256
D, H, HD, FF, V, L = 512, 8, 64, 2048, 32000, 2
EPS = 1e-5
P = 128
TB = T // P            # 4 token blocks
DB = D // P            # 4 feature blocks
FB = FF // P           # 16 ffn blocks
VN = 512               # vocab tile width
VBF = V // VN          # 62 full vocab blocks
VREM = V - VBF * VN    # 256 remainder
NCORES = 8
INV_SQRT_HD = 1.0 / math.sqrt(HD)

_NC_CACHE = {}


def _quantize_weight(W):
    """Host-side eval-mode BitNet weight quantization (matches reference)."""
    W = np.asarray(W, dtype=np.float32)
    w_scale = np.maximum(np.abs(W).mean(dtype=np.float32), np.float32(1e-5))
    qW = np.sign(W) * (np.abs(W) > np.float32(0.5) * w_scale)
    qWT = np.ascontiguousarray(qW.astype(np.float32).T.astype(ml_dtypes.bfloat16))
    return qWT, float(w_scale)


def _col_layout(v):
    """[N] -> [128, N/128]: element [p, j] = v[j*128 + p] (per-partition cols)."""
    v = np.asarray(v, dtype=np.float32)
    n = v.shape[0]
    return np.ascontiguousarray(v.reshape(n // P, P).T)


def build(wscales, debug=False):
    nc = bacc.Bacc("TRN2", target_bir_lowering=False, debug=False,
                   num_devices=NCORES)

    x_in = nc.dram_tensor("x", [T, D], F32, kind="ExternalInput")
    enc_in = nc.dram_tensor("enc", [B * S, D], F32, kind="ExternalInput")
    tri_in = nc.dram_tensor("tri", [P, P], F32, kind="ExternalInput")

    wq, wk, wv, wo, wenc, w1, w2 = [], [], [], [], [], [], []
    bq, bk, b1 = [], [], []
    for l in range(L):
        wq.append(nc.dram_tensor(f"wq{l}", [D, D], BF16, kind="ExternalInput"))
        wk.append(nc.dram_tensor(f"wk{l}", [D, D], BF16, kind="ExternalInput"))
        wv.append(nc.dram_tensor(f"wv{l}", [D, D], BF16, kind="ExternalInput"))
        wo.append(nc.dram_tensor(f"wo{l}", [D, D], BF16, kind="ExternalInput"))
        wenc.append(nc.dram_tensor(f"wenc{l}", [D, D], BF16, kind="ExternalInput"))
        w1.append(nc.dram_tensor(f"w1{l}", [D, FF], BF16, kind="ExternalInput"))
        w2.append(nc.dram_tensor(f"w2{l}", [FF, D], BF16, kind="ExternalInput"))
        bq.append(nc.dram_tensor(f"bq{l}", [P, DB], F32, kind="ExternalInput"))
        bk.append(nc.dram_tensor(f"bk{l}", [P, DB], F32, kind="ExternalInput"))
        b1.append(nc.dram_tensor(f"b1{l}", [P, FB], F32, kind="ExternalInput"))
    wout_in = nc.dram_tensor("wout", [D, V], I8, kind="ExternalInput")

    out = nc.dram_tensor("logits", [T, V], BF16, kind="ExternalOutput")

    dbg = {}
    if debug:
        for name, shape in [("dbg_h0", [T, D]), ("dbg_h1", [T, D]),
                            ("dbg_ln1", [T, D]), ("dbg_attn", [D, T]),
                            ("dbg_qT", [D, T]), ("dbg_gelu", [FF, T])]:
            dbg[name] = nc.dram_tensor(name, shape, F32, kind="ExternalOutput")

    with tile.TileContext(nc) as tc:
        _body(nc, tc, wscales, x_in, enc_in, tri_in,
              wq, wk, wv, wo, wenc, w1, w2, bq, bk, b1, wout_in, out, dbg)
    nc.compile()
    return nc


def _body(nc, tc, ws, x_in, enc_in, tri_in,
          wq, wk, wv, wo, wenc, w1, w2, bq_t, bk_t, b1_t, wout_in, out, dbg):
    from contextlib import ExitStack
    ctx = ExitStack()
    with ctx:
        # ---------------- pools (uniform tile shape per pool) ----------------
        hp = ctx.enter_context(tc.tile_pool(name="hp", bufs=TB))
        const = ctx.enter_context(tc.tile_pool(name="const", bufs=1))
        lnp = ctx.enter_context(tc.tile_pool(name="lnp", bufs=TB))
        stat = ctx.enter_context(tc.tile_pool(name="stat", bufs=2))
        colp = ctx.enter_context(tc.tile_pool(name="colp", bufs=2))
        qxp = ctx.enter_context(tc.tile_pool(name="qxp", bufs=3))
        qtp = ctx.enter_context(tc.tile_pool(name="qtp", bufs=9))
        wsp = ctx.enter_context(tc.tile_pool(name="wsp", bufs=12))
        w1p = ctx.enter_context(tc.tile_pool(name="w1p", bufs=DB))
        w2p = ctx.enter_context(tc.tile_pool(name="w2p", bufs=FB))
        qkp = ctx.enter_context(tc.tile_pool(name="qkp", bufs=2 * DB))
        vp = ctx.enter_context(tc.tile_pool(name="vp", bufs=TB))
        vhp = ctx.enter_context(tc.tile_pool(name="vhp", bufs=3))
        expp = ctx.enter_context(tc.tile_pool(name="expp", bufs=3))
        atp = ctx.enter_context(tc.tile_pool(name="atp", bufs=TB))
        gelp = ctx.enter_context(tc.tile_pool(name="gelp", bufs=FB))
        qgp = ctx.enter_context(tc.tile_pool(name="qgp", bufs=FB))
        drp = ctx.enter_context(tc.tile_pool(name="drp", bufs=4, space="DRAM"))
        outp = ctx.enter_context(tc.tile_pool(name="outp", bufs=4))
        wop = ctx.enter_context(tc.tile_pool(name="wop", bufs=3))
        ps = ctx.enter_context(tc.tile_pool(name="ps", bufs=5, space="PSUM"))
        pst_p = ctx.enter_context(tc.tile_pool(name="pst", bufs=1, space="PSUM"))
        pso = ctx.enter_context(tc.tile_pool(name="pso", bufs=2, space="PSUM"))

        # ---------------- constants ----------------
        tri = const.tile([P, P], F32)
        nc.sync.dma_start(out=tri[:], in_=tri_in[:])
        eps_c = const.tile([P, 1], F32)
        nc.vector.memset(eps_c[:], EPS)
        ones_row = const.tile([1, P], F32)
        nc.vector.memset(ones_row[:], 1.0)
        ones_col = const.tile([P, 1], F32)
        nc.vector.memset(ones_col[:], 1.0)
        ident_bf = const.tile([P, P], BF16)
        make_identity(nc, ident_bf)

        # ---------------- load x into resident h tiles ----------------
        h = []
        for tb in range(TB):
            ht = hp.tile([P, D], F32, tag="h")
            nc.sync.dma_start(out=ht[:], in_=x_in[tb * P:(tb + 1) * P, :])
            h.append(ht)

        # =========== helpers ===========
        def cross_part_absmax(tiles, tag):
            """tiles: list of [128, n] f32 -> [128,1] all-partition absmax
            (batch-local global max), clamped at 1e-8."""
            acc = None
            for tl in tiles:
                c = colp.tile([P, 1], F32, tag="col", bufs=12)
                nc.vector.tensor_reduce(out=c[:], in_=tl[:], axis=AX.X,
                                        op=ALU.max, apply_absolute_value=True)
                if acc is None:
                    acc = c
                else:
                    nxt = colp.tile([P, 1], F32, tag="col", bufs=12)
                    nc.vector.tensor_max(out=nxt[:], in0=acc[:], in1=c[:])
                    acc = nxt
            m1 = stat.tile([1, 1], F32, tag="cpm1")
            nc.gpsimd.tensor_reduce(out=m1[:], in_=acc[:], axis=AX.C, op=ALU.max)
            mb = colp.tile([P, 1], F32, tag="mhat", bufs=4)
            nc.gpsimd.partition_broadcast(mb[:], m1[:], channels=P)
            mc = colp.tile([P, 1], F32, tag="mhat", bufs=4)
            nc.vector.tensor_scalar_max(out=mc[:], in0=mb[:], scalar1=1e-8)
            return mc

        def iscale_of(mhat, tag):
            # returns INVERSE scale 127/m (quantize multiplies by this)
            rcp = colp.tile([P, 1], F32, tag="isc", bufs=6)
            nc.vector.reciprocal(out=rcp[:], in_=mhat[:])
            inv = colp.tile([P, 1], F32, tag="isc", bufs=6)
            nc.vector.tensor_scalar_mul(out=inv[:], in0=rcp[:], scalar1=127.0)
            return inv

        def quantize_tiles(src_tiles, isc, n=D):
            """f32 [128,n] tiles -> bf16 [128,n] tiles (int8 round/saturate)."""
            res = []
            for st in src_tiles:
                q8 = qxp.tile([P, T], I8, tag="q8")
                nc.vector.tensor_scalar_mul(out=q8[:, :n], in0=st[:], scalar1=isc[:])
                qb = qtp.tile([P, T], BF16, tag="qt")
                nc.gpsimd.tensor_copy(out=qb[:, :n], in_=q8[:, :n])
                res.append(qb)
            return res

        def transpose_to(qtiles):
            """TB x [128, D] bf16 (rows=t) -> DB x [128, T] bf16 (rows=d)."""
            outt = [qtp.tile([P, T], BF16, tag="qt", name="qt") for _ in range(DB)]
            for tb in range(TB):
                for db in range(DB):
                    pst = pst_p.tile([P, P], BF16, tag="pstr", name="pstr")
                    nc.tensor.transpose(pst[:P, :P],
                                        qtiles[tb][:, db * P:(db + 1) * P],
                                        ident_bf[:])
                    nc.vector.tensor_copy(out=outt[db][:, tb * P:(tb + 1) * P],
                                          in_=pst[:P, :P])
            return outt

        def layer_norm(tiles):
            outs = []
            for tb in range(TB):
                st6 = stat.tile([P, 6], F32, tag="bn6")
                nc.vector.bn_stats(out=st6[:], in_=tiles[tb][:])
                mv = stat.tile([P, 2], F32, tag="bn2")
                nc.vector.bn_aggr(out=mv[:], in_=st6[:])
                std = colp.tile([P, 1], F32, tag="col", bufs=12)
                nc.scalar.activation(out=std[:], in_=mv[:, 1:2], func=ACTF.Sqrt,
                                     bias=eps_c[:], scale=1.0)
                rstd = colp.tile([P, 1], F32, tag="col", bufs=12)
                nc.vector.reciprocal(out=rstd[:], in_=std[:])
                ot = lnp.tile([P, D], F32, tag="ln")
                nc.vector.tensor_scalar(out=ot[:], in0=tiles[tb][:],
                                        scalar1=mv[:, 0:1], scalar2=rstd[:],
                                        op0=ALU.subtract, op1=ALU.mult)
                outs.append(ot)
            return outs

        # ---------------- ctx prep (encoder mean + quantize, once) ----------
        ctx_rows = []
        for b in range(B):
            pctx = ps.tile([P, VN], F32, tag="psum")
            for sb in range(S // P):
                et = lnp.tile([P, D], F32, tag="ln")
                nc.sync.dma_start(
                    out=et[:], in_=enc_in[b * S + sb * P: b * S + (sb + 1) * P, :])
                nc.tensor.matmul(pctx[:1, :D], ones_col[:], et[:],
                                 start=(sb == 0), stop=(sb == S // P - 1))
            cr = stat.tile([1, D], F32, tag="ctxr", bufs=B)
            nc.scalar.activation(out=cr[:], in_=pctx[:1, :D],
                                 func=ACTF.Copy, scale=1.0 / S)
            ctx_rows.append(cr)
        cacc = None
        for b in range(B):
            cm = stat.tile([1, 1], F32, tag="cm", bufs=4)
            nc.vector.tensor_reduce(out=cm[:], in_=ctx_rows[b][:], axis=AX.X,
                                    op=ALU.max, apply_absolute_value=True)
            if cacc is None:
                cacc = cm
            else:
                nx = stat.tile([1, 1], F32, tag="cm", bufs=4)
                nc.vector.tensor_max(out=nx[:], in0=cacc[:], in1=cm[:])
                cacc = nx
        cmb = colp.tile([P, 1], F32, tag="ctxm")
        nc.gpsimd.partition_broadcast(cmb[:], cacc[:], channels=P)
        cmc = colp.tile([P, 1], F32, tag="ctxm")
        nc.vector.tensor_scalar_max(out=cmc[:], in0=cmb[:], scalar1=1e-8)
        isc_ctx = iscale_of(cmc, "ctx")
        q8row = stat.tile([1, D], I8, tag="q8ctx")
        nc.vector.tensor_scalar_mul(out=q8row[:], in0=ctx_rows[0][:],
                                    scalar1=isc_ctx[:1, :])
        dctx8 = drp.tile([1, D], I8, tag="dctx8")
        nc.sync.dma_start(out=dctx8[:], in_=q8row[:])
        qctx8 = stat.tile([P, DB], I8, tag="qctx8")
        nc.sync.dma_start(out=qctx8[:],
                          in_=dctx8[:].rearrange("one (j p) -> p (one j)", p=P))
        qctxT = const.tile([P, DB], BF16)
        nc.scalar.copy(out=qctxT[:], in_=qctx8[:])

        # ---------------- transformer layers ----------------
        for l in range(L):
            wsq, wsk, wsv, wso, wse, ws1, ws2 = ws[l]

            wq_sb = [wsp.tile([P, D], BF16, tag="ws", name="ws") for _ in range(DB)]
            wk_sb = [wsp.tile([P, D], BF16, tag="ws", name="ws") for _ in range(DB)]
            wv_sb = [wsp.tile([P, D], BF16, tag="ws", name="ws") for _ in range(DB)]
            for db in range(DB):
                nc.sync.dma_start(out=wq_sb[db][:], in_=wq[l][db * P:(db + 1) * P, :])
                nc.sync.dma_start(out=wk_sb[db][:], in_=wk[l][db * P:(db + 1) * P, :])
                nc.sync.dma_start(out=wv_sb[db][:], in_=wv[l][db * P:(db + 1) * P, :])
            bq_sb = const.tile([P, DB], F32, tag="bqc")
            bk_sb = const.tile([P, DB], F32, tag="bkc")
            nc.sync.dma_start(out=bq_sb[:], in_=bq_t[l][:])
            nc.sync.dma_start(out=bk_sb[:], in_=bk_t[l][:])

            # --- ln1 + quantize + transpose ---
            ln1 = layer_norm(h)
            if dbg and l == 0:
                for tb in range(TB):
                    nc.sync.dma_start(out=dbg["dbg_ln1"][tb * P:(tb + 1) * P, :],
                                      in_=ln1[tb][:])
            m_ln1 = cross_part_absmax(ln1, f"ln1_{l}")
            isc1 = iscale_of(m_ln1, f"ln1_{l}")
            q1 = quantize_tiles(ln1, isc1)
            q1T = transpose_to(q1)

            # --- qkv matmuls ---
            sc_q = colp.tile([P, 1], F32, tag="sc", bufs=6)
            nc.scalar.mul(out=sc_q[:], in_=m_ln1[:], mul=wsq * INV_SQRT_HD / 127.0)
            sc_k = colp.tile([P, 1], F32, tag="sc", bufs=6)
            nc.scalar.mul(out=sc_k[:], in_=m_ln1[:], mul=wsk / 127.0)
            sc_v = colp.tile([P, 1], F32, tag="sc", bufs=6)
            nc.scalar.mul(out=sc_v[:], in_=m_ln1[:], mul=wsv / 127.0)
            qT, kT = [], []
            for ob in range(DB):
                pq = ps.tile([P, VN], F32, tag="psum")
                for db in range(DB):
                    nc.tensor.matmul(pq[:, :T], wq_sb[db][:, ob * P:(ob + 1) * P],
                                     q1T[db][:], start=(db == 0), stop=(db == DB - 1))
                qf = qkp.tile([P, T], F32, tag="qk")
                nc.scalar.activation(out=qf[:], in_=pq[:, :T], func=ACTF.Identity,
                                     bias=bq_sb[:, ob:ob + 1], scale=sc_q[:])
                qT.append(qf)
                pk = ps.tile([P, VN], F32, tag="psum")
                for db in range(DB):
                    nc.tensor.matmul(pk[:, :T], wk_sb[db][:, ob * P:(ob + 1) * P],
                                     q1T[db][:], start=(db == 0), stop=(db == DB - 1))
                kf = qkp.tile([P, T], F32, tag="qk")
                nc.scalar.activation(out=kf[:], in_=pk[:, :T], func=ACTF.Identity,
                                     bias=bk_sb[:, ob:ob + 1], scale=sc_k[:])
                kT.append(kf)
            if dbg and l == 0:
                for ob in range(DB):
                    nc.sync.dma_start(out=dbg["dbg_qT"][ob * P:(ob + 1) * P, :],
                                      in_=qT[ob][:])
            v_sb = []
            for tb in range(TB):
                pv = ps.tile([P, VN], F32, tag="psum")
                for db in range(DB):
                    nc.tensor.matmul(pv[:, :D], q1T[db][:, tb * P:(tb + 1) * P],
                                     wv_sb[db][:], start=(db == 0), stop=(db == DB - 1))
                vt = vp.tile([P, H, HD + 1], F32, tag="v")
                nc.scalar.activation(
                    out=vt[:, :, :HD],
                    in_=pv[:, :D].rearrange("p (h d) -> p h d", h=H),
                    func=ACTF.Identity, bias=0.0, scale=sc_v[:])
                nc.vector.memset(vt[:, :, HD:HD + 1], 1.0)
                v_sb.append(vt)

            # --- attention (fp32) ---
            attnT = [atp.tile([P, T], F32, tag="at", name="at") for _ in range(DB)]
            for hh in range(H):
                ob, off = hh // 2, (hh % 2) * HD
                po = pso.tile([P, VN], F32, tag="psumo")
                for kb in range(TB):
                    qoff = kb * P
                    pss = ps.tile([P, VN], F32, tag="psum")
                    nc.tensor.matmul(
                        pss[:, qoff:T],
                        kT[ob][off:off + HD, kb * P:(kb + 1) * P],
                        qT[ob][off:off + HD, qoff:T],
                        start=True, stop=True)
                    mrow = colp.tile([P, 1], F32, tag="mrow", bufs=6)
                    nc.vector.tensor_reduce(out=mrow[:], in_=pss[:, qoff:T],
                                            axis=AX.X, op=ALU.max)
                    nmrow = colp.tile([P, 1], F32, tag="mrow", bufs=6)
                    nc.scalar.mul(out=nmrow[:], in_=mrow[:], mul=-1.0)
                    emrow = colp.tile([P, 1], F32, tag="mrow", bufs=6)
                    nc.scalar.activation(out=emrow[:], in_=mrow[:], func=ACTF.Exp)
                    ex = expp.tile([P, T], F32, tag="exp")
                    if qoff:
                        nc.gpsimd.memset(ex[:, :qoff], 0.0)
                    nc.scalar.activation(out=ex[:, qoff:T], in_=pss[:, qoff:T],
                                         func=ACTF.Exp, bias=nmrow[:], scale=1.0)
                    nc.vector.tensor_mul(out=ex[:, qoff:qoff + P],
                                         in0=ex[:, qoff:qoff + P], in1=tri[:])
                    vh = vhp.tile([P, HD + 1], F32, tag="vh")
                    nc.vector.tensor_scalar_mul(
                        out=vh[:], in0=v_sb[kb][:, hh, :], scalar1=emrow[:])
                    nc.tensor.matmul(po[:HD + 1, qoff:T], vh[:], ex[:, qoff:T],
                                     start=(kb == 0), stop=(kb == TB - 1))
                rec = stat.tile([1, T], F32, tag="rec")
                nc.vector.reciprocal(out=rec[:], in_=po[HD:HD + 1, :T])
                pb = ps.tile([P, VN], F32, tag="psum")
                nc.tensor.matmul(pb[:HD, :T], ones_row[:1, :HD], rec[:],
                                 start=True, stop=True)
                nc.scalar.copy(out=attnT[ob][off:off + HD, :], in_=po[:HD, :T])
                nc.vector.tensor_mul(out=attnT[ob][off:off + HD, :],
                                     in0=attnT[ob][off:off + HD, :],
                                     in1=pb[:HD, :T])
            if dbg and l == 0:
                for ob in range(DB):
                    nc.sync.dma_start(out=dbg["dbg_attn"][ob * P:(ob + 1) * P, :],
                                      in_=attnT[ob][:])

            # --- attention output projection (Wo) + residual ---
            wo_sb = [wsp.tile([P, D], BF16, tag="ws", name="ws") for _ in range(DB)]
            we_sb = [wsp.tile([P, D], BF16, tag="ws", name="ws") for _ in range(DB)]
            for db in range(DB):
                nc.sync.dma_start(out=wo_sb[db][:], in_=wo[l][db * P:(db + 1) * P, :])
                nc.sync.dma_start(out=we_sb[db][:], in_=wenc[l][db * P:(db + 1) * P, :])
            m_o = cross_part_absmax(attnT, f"o_{l}")
            isc_o = iscale_of(m_o, f"o_{l}")
            qo = quantize_tiles(attnT, isc_o, n=T)
            sc_o = colp.tile([P, 1], F32, tag="sc", bufs=6)
            nc.scalar.mul(out=sc_o[:], in_=m_o[:], mul=wso / 127.0)
            for tb in range(TB):
                pw = ps.tile([P, VN], F32, tag="psum")
                for ob in range(DB):
                    nc.tensor.matmul(pw[:, :D], qo[ob][:, tb * P:(tb + 1) * P],
                                     wo_sb[ob][:], start=(ob == 0), stop=(ob == DB - 1))
                nc.vector.scalar_tensor_tensor(out=h[tb][:], in0=pw[:, :D],
                                               scalar=sc_o[:], in1=h[tb][:],
                                               op0=ALU.mult, op1=ALU.add)

            # --- encoder-context projection + residual (broadcast over t) ---
            pe = ps.tile([P, VN], F32, tag="psum")
            for db in range(DB):
                nc.tensor.matmul(pe[:1, :D], qctxT[:, db:db + 1], we_sb[db][:],
                                 start=(db == 0), stop=(db == DB - 1))
            enc_row = stat.tile([1, D], F32, tag="encrow", bufs=1)
            sc_e = stat.tile([1, 1], F32, tag="sc_e")
            nc.scalar.mul(out=sc_e[:], in_=cmc[:1, :], mul=wse / 127.0)
            nc.scalar.activation(out=enc_row[:], in_=pe[:1, :D],
                                 func=ACTF.Copy, scale=sc_e[:])
            pbe = ps.tile([P, VN], F32, tag="psum")
            nc.tensor.matmul(pbe[:, :D], ones_row[:1, :P], enc_row[:],
                             start=True, stop=True)
            for tb in range(TB):
                nc.vector.tensor_add(out=h[tb][:], in0=h[tb][:], in1=pbe[:, :D])

            # --- FFN ---
            ln3 = layer_norm(h)
            m_ln3 = cross_part_absmax(ln3, f"ln3_{l}")
            isc3 = iscale_of(m_ln3, f"ln3_{l}")
            q3 = quantize_tiles(ln3, isc3)
            q3T = transpose_to(q3)
            w1_sb = [w1p.tile([P, FF], BF16, tag="w1", name="w1") for _ in range(DB)]
            for db in range(DB):
                nc.sync.dma_start(out=w1_sb[db][:], in_=w1[l][db * P:(db + 1) * P, :])
            b1_sb = const.tile([P, FB], F32, tag="b1c")
            nc.sync.dma_start(out=b1_sb[:], in_=b1_t[l][:])
            sc_1 = colp.tile([P, 1], F32, tag="sc", bufs=6)
            nc.scalar.mul(out=sc_1[:], in_=m_ln3[:], mul=ws1 / 127.0)
            gel = []
            for fb in range(FB):
                pf = ps.tile([P, VN], F32, tag="psum")
                for db in range(DB):
                    nc.tensor.matmul(pf[:, :T], w1_sb[db][:, fb * P:(fb + 1) * P],
                                     q3T[db][:], start=(db == 0), stop=(db == DB - 1))
                gt = gelp.tile([P, T], F32, tag="gel")
                nc.scalar.activation(out=gt[:], in_=pf[:, :T], func=ACTF.Gelu,
                                     bias=b1_sb[:, fb:fb + 1], scale=sc_1[:])
                gel.append(gt)
            if dbg and l == 0:
                for fb in range(FB):
                    nc.sync.dma_start(out=dbg["dbg_gelu"][fb * P:(fb + 1) * P, :],
                                      in_=gel[fb][:])
            m_g = cross_part_absmax(gel, f"g_{l}")
            isc_g = iscale_of(m_g, f"g_{l}")
            sc_2 = colp.tile([P, 1], F32, tag="sc", bufs=6)
            nc.scalar.mul(out=sc_2[:], in_=m_g[:], mul=ws2 / 127.0)
            qg = []
            for fb in range(FB):
                g8 = qxp.tile([P, T], I8, tag="q8")
                nc.vector.tensor_scalar_mul(out=g8[:], in0=gel[fb][:], scalar1=isc_g[:])
                gq = qgp.tile([P, T], BF16, tag="qg")
                nc.gpsimd.tensor_copy(out=gq[:], in_=g8[:])
                qg.append(gq)
            w2_sb = [w2p.tile([P, D], BF16, tag="w2", name="w2") for _ in range(FB)]
            for fb in range(FB):
                nc.sync.dma_start(out=w2_sb[fb][:], in_=w2[l][fb * P:(fb + 1) * P, :])
            for tb in range(TB):
                pf2 = ps.tile([P, VN], F32, tag="psum")
                for fb in range(FB):
                    nc.tensor.matmul(pf2[:, :D], qg[fb][:, tb * P:(tb + 1) * P],
                                     w2_sb[fb][:], start=(fb == 0), stop=(fb == FB - 1))
                nc.vector.scalar_tensor_tensor(out=h[tb][:], in0=pf2[:, :D],
                                               scalar=sc_2[:], in1=h[tb][:],
                                               op0=ALU.mult, op1=ALU.add)
            if dbg:
                tgt = dbg["dbg_h0"] if l == 0 else dbg["dbg_h1"]
                for tb in range(TB):
                    nc.sync.dma_start(out=tgt[tb * P:(tb + 1) * P, :], in_=h[tb][:])

        # ---------------- final LN + output projection ----------------
        ws_out = ws[L]
        lnf = layer_norm(h)
        m_h = cross_part_absmax(lnf, "lnf")
        isc_h = iscale_of(m_h, "lnf")
        qh = quantize_tiles(lnf, isc_h)
        qhT = transpose_to(qh)
        sc_out = colp.tile([P, 1], F32, tag="sc_out")
        nc.scalar.mul(out=sc_out[:], in_=m_h[:], mul=ws_out / 127.0)
        nvb = VBF + (1 if VREM else 0)
        # process vocab blocks in pairs: one [128, 1024] bf16 staging tile per
        # (tb, pair) -> bigger, fewer output DMAs
        pair_starts = list(range(0, VBF - 1, 2))       # (0,1), (2,3), ... (60,61)
        tail = [VBF] if VREM else []                   # remainder block alone
        evict_i = 0
        for pv, v0 in enumerate(pair_starts + tail):
            pair = v0 < VBF - 1
            vws = [(v0, VN), (v0 + 1, VN)] if pair else [(v0, VREM)]
            w8 = [wop.tile([P, 2 * VN], I8, tag="w8", name="w8", bufs=5)
                  for _ in range(DB)]
            wtile = [wop.tile([P, 2 * VN], BF16, tag="wout", name="wout", bufs=5)
                     for _ in range(DB)]
            wn_tot = sum(vn for _, vn in vws)
            for db in range(DB):
                nc.sync.dma_start(out=w8[db][:, :wn_tot],
                                    in_=wout_in[db * P:(db + 1) * P,
                                                v0 * VN:v0 * VN + wn_tot])
                nc.gpsimd.tensor_copy(out=wtile[db][:, :wn_tot],
                                      in_=w8[db][:, :wn_tot])
            for tb in range(TB):
                lt = outp.tile([P, 2 * VN], BF16, tag="lt")
                for si, (vb, vn) in enumerate(vws):
                    pl = ps.tile([P, VN], F32, tag="psum")
                    for db in range(DB):
                        nc.tensor.matmul(
                            pl[:, :vn],
                            qhT[db][:, tb * P:(tb + 1) * P],
                            wtile[db][:, si * VN:si * VN + vn],
                            start=(db == 0), stop=(db == DB - 1))
                    if evict_i % 2 == 0:
                        nc.scalar.activation(out=lt[:, si * VN:si * VN + vn],
                                             in_=pl[:, :vn], func=ACTF.Copy,
                                             scale=sc_out[:])
                    else:
                        nc.vector.tensor_scalar_mul(
                            out=lt[:, si * VN:si * VN + vn], in0=pl[:, :vn],
                            scalar1=sc_out[:])
                    evict_i += 1
                eng = nc.sync if (tb % 2 == 0) else nc.scalar
                eng.dma_start(
                    out=out[tb * P:(tb + 1) * P, v0 * VN:v0 * VN + wn_tot],
                    in_=lt[:, :wn_tot])


def _prep(inputs):
    """Host-side packing shared across cores; returns (common dict, wscales)."""
    f32 = np.float32
    ws = []
    common = {}
    for l in range(L):
        packed = []
        for name, W in [("wq", inputs["Wq"][l]), ("wk", inputs["Wk"][l]),
                        ("wv", inputs["Wv"][l]), ("wo", inputs["Wo"][l]),
                        ("wenc", inputs["Wenc"][l]), ("w1", inputs["W1"][l]),
                        ("w2", inputs["W2"][l])]:
            qWT, s = _quantize_weight(W)
            common[f"{name}{l}"] = qWT
            packed.append(s)
        ws.append(tuple(packed))
        common[f"bq{l}"] = _col_layout(
            np.asarray(inputs["bq"][l], f32) * f32(INV_SQRT_HD))
        common[f"bk{l}"] = _col_layout(inputs["bk"][l])
        common[f"b1{l}"] = _col_layout(inputs["b1"][l])
    qWoutT, s_out = _quantize_weight(inputs["Wout"])
    common["wout"] = np.ascontiguousarray(
        np.asarray(qWoutT, np.float32).astype(np.int8))
    ws.append(s_out)
    common["tri"] = np.triu(np.ones((P, P), dtype=f32))
    return common, ws


def kernel(**inputs):
    debug = bool(int(os.environ.get("BITGEN_DEBUG", "0")))
    common, ws = _prep(inputs)

    # Fast path assumes the model's declared fills: zero biases on the layers
    # without per-partition bias layout, identity LN affines.
    for nm in ["bo", "benc", "b2", "bout", "bv",
               "ln1b", "ln2b", "ln3b", "lnfb"]:
        assert not np.any(np.asarray(inputs[nm])), f"nonzero {nm} unsupported"
    for nm in ["ln1g", "ln2g", "ln3g", "lnfg"]:
        assert np.all(np.asarray(inputs[nm]) == 1.0), f"non-unit {nm} unsupported"

    key = ("v1", debug, tuple(np.asarray(w, np.float64).tobytes()
                              for w in (tuple(ws[l]) for l in range(L)))), ws[L]
    key = (repr(ws), debug)
    if key not in _NC_CACHE:
        _NC_CACHE[key] = build(ws, debug=debug)
    nc = _NC_CACHE[key]

    enc = np.asarray(inputs["encoder_output"], np.float32)
    x = np.asarray(inputs["x"], np.float32)
    in_maps = []
    for c in range(NCORES):
        m = dict(common)
        m["x"] = np.ascontiguousarray(x[c])
        # roll so this core's batch is the first S-row block (the kernel
        # quantizes ctx row 0 as its own batch, using all rows for the scale)
        m["enc"] = np.ascontiguousarray(
            np.roll(enc, -c, axis=0).reshape(B * S, D))
        in_maps.append(m)

    res = run_bass_kernel_spmd(nc, in_maps, list(range(NCORES)))
    outs = [res.results[c]["logits"].astype(np.float32) for c in range(NCORES)]
    return np.stack(outs, axis=0)


# revision 15
# speedup vs baseline: 1.2766x; 1.0677x over previous
"""Trainium2 Bass kernel for nn_BitGenModel (BitNet-style dense transformer).

Sharding: data-parallel over batch (B=8) across 8 NeuronCores; each core runs
the full 2-layer transformer + final 32000-vocab projection for its batch
element. Weights are pre-quantized (ternary, bf16) and pre-transposed on the
host; activation quantization (per-tensor absmax int8) runs on device with
batch-local scales.

Numerics:
- BitLinear integer matmuls run on the PE in bf16: |qx|<=127 and qW in
  {-1,0,1} are exactly representable and accumulation is fp32 in PSUM, so
  these matmuls are exact.
- Attention (scores, softmax, A@V) runs in fp32. Softmax subtracts a
  per-key-row max m_k and folds an exp(m_k) correction column into the V
  operand; the m_k cancel exactly in the normalization, so no cross-partition
  reductions are needed.
- LayerNorm uses DVE bn_stats/bn_aggr (fp32, free-dim reduction) in the
  natural [token, feature] layout.
"""
import math
import os

import numpy as np
import ml_dtypes

import concourse.bass as bass
import concourse.mybir as mybir
import concourse.tile as tile
from concourse import bacc
from concourse.bass_utils import run_bass_kernel_spmd
from concourse.masks import make_identity

F32 = mybir.dt.float32
BF16 = mybir.dt.bfloat16
I8 = mybir.dt.int8
AX = mybir.AxisListType
ALU = mybir.AluOpType
ACTF = mybir.ActivationFunctionType

B, T, S = 8, 512, 256
D, H, HD, FF, V, L = 512, 8, 64, 2048, 32000, 2
EPS = 1e-5
P = 128
TB = T // P            # 4 token blocks
DB = D // P            # 4 feature blocks
FB = FF // P           # 16 ffn blocks
VN = 512               # vocab tile width
VBF = V // VN          # 62 full vocab blocks
VREM = V - VBF * VN    # 256 remainder
NCORES = 8
INV_SQRT_HD = 1.0 / math.sqrt(HD)

_NC_CACHE = {}


def _quantize_weight(W):
    """Host-side eval-mode BitNet weight quantization (matches reference)."""
    W = np.asarray(W, dtype=np.float32)
    w_scale = np.maximum(np.abs(W).mean(dtype=np.float32), np.float32(1e-5))
    qW = np.sign(W) * (np.abs(W) > np.float32(0.5) * w_scale)
    qWT = np.ascontiguousarray(qW.astype(np.float32).T.astype(ml_dtypes.bfloat16))
    return qWT, float(w_scale)


def _col_layout(v):
    """[N] -> [128, N/128]: element [p, j] = v[j*128 + p] (per-partition cols)."""
    v = np.asarray(v, dtype=np.float32)
    n = v.shape[0]
    return np.ascontiguousarray(v.reshape(n // P, P).T)


def build(wscales, debug=False):
    nc = bacc.Bacc("TRN2", target_bir_lowering=False, debug=False,
                   num_devices=NCORES)

    x_in = nc.dram_tensor("x", [T, D], F32, kind="ExternalInput")
    enc_in = nc.dram_tensor("enc", [B * S, D], F32, kind="ExternalInput")
    tri_in = nc.dram_tensor("tri", [P, P], F32, kind="ExternalInput")

    wq, wk, wv, wo, wenc, w1, w2 = [], [], [], [], [], [], []
    bq, bk, b1 = [], [], []
    for l in range(L):
        wq.append(nc.dram_tensor(f"wq{l}", [D, D], BF16, kind="ExternalInput"))
        wk.append(nc.dram_tensor(f"wk{l}", [D, D], BF16, kind="ExternalInput"))
        wv.append(nc.dram_tensor(f"wv{l}", [D, D], BF16, kind="ExternalInput"))
        wo.append(nc.dram_tensor(f"wo{l}", [D, D], BF16, kind="ExternalInput"))
        wenc.append(nc.dram_tensor(f"wenc{l}", [D, D], BF16, kind="ExternalInput"))
        w1.append(nc.dram_tensor(f"w1{l}", [D, FF], BF16, kind="ExternalInput"))
        w2.append(nc.dram_tensor(f"w2{l}", [FF, D], BF16, kind="ExternalInput"))
        bq.append(nc.dram_tensor(f"bq{l}", [P, DB], F32, kind="ExternalInput"))
        bk.append(nc.dram_tensor(f"bk{l}", [P, DB], F32, kind="ExternalInput"))
        b1.append(nc.dram_tensor(f"b1{l}", [P, FB], F32, kind="ExternalInput"))
    wout_in = nc.dram_tensor("wout", [D, V], I8, kind="ExternalInput")

    out = nc.dram_tensor("logits", [T, V], BF16, kind="ExternalOutput")

    dbg = {}
    if debug:
        for name, shape in [("dbg_h0", [T, D]), ("dbg_h1", [T, D]),
                            ("dbg_ln1", [T, D]), ("dbg_attn", [D, T]),
                            ("dbg_qT", [D, T]), ("dbg_gelu", [FF, T])]:
            dbg[name] = nc.dram_tensor(name, shape, F32, kind="ExternalOutput")

    with tile.TileContext(nc) as tc:
        _body(nc, tc, wscales, x_in, enc_in, tri_in,
              wq, wk, wv, wo, wenc, w1, w2, bq, bk, b1, wout_in, out, dbg)
    nc.compile()
    return nc


def _body(nc, tc, ws, x_in, enc_in, tri_in,
          wq, wk, wv, wo, wenc, w1, w2, bq_t, bk_t, b1_t, wout_in, out, dbg):
    from contextlib import ExitStack
    ctx = ExitStack()
    with ctx:
        # ---------------- pools (uniform tile shape per pool) ----------------
        hp = ctx.enter_context(tc.tile_pool(name="hp", bufs=TB))
        const = ctx.enter_context(tc.tile_pool(name="const", bufs=1))
        lnp = ctx.enter_context(tc.tile_pool(name="lnp", bufs=TB))
        stat = ctx.enter_context(tc.tile_pool(name="stat", bufs=2))
        colp = ctx.enter_context(tc.tile_pool(name="colp", bufs=2))
        qxp = ctx.enter_context(tc.tile_pool(name="qxp", bufs=3))
        qtp = ctx.enter_context(tc.tile_pool(name="qtp", bufs=9))
        wsp = ctx.enter_context(tc.tile_pool(name="wsp", bufs=12))
        w1p = ctx.enter_context(tc.tile_pool(name="w1p", bufs=DB))
        w2p = ctx.enter_context(tc.tile_pool(name="w2p", bufs=FB))
        qkp = ctx.enter_context(tc.tile_pool(name="qkp", bufs=2 * DB))
        vp = ctx.enter_context(tc.tile_pool(name="vp", bufs=TB))
        vhp = ctx.enter_context(tc.tile_pool(name="vhp", bufs=3))
        expp = ctx.enter_context(tc.tile_pool(name="expp", bufs=3))
        atp = ctx.enter_context(tc.tile_pool(name="atp", bufs=TB))
        gelp = ctx.enter_context(tc.tile_pool(name="gelp", bufs=FB))
        qgp = ctx.enter_context(tc.tile_pool(name="qgp", bufs=FB))
        drp = ctx.enter_context(tc.tile_pool(name="drp", bufs=4, space="DRAM"))
        outp = ctx.enter_context(tc.tile_pool(name="outp", bufs=4))
        wop = ctx.enter_context(tc.tile_pool(name="wop", bufs=3))
        ps = ctx.enter_context(tc.tile_pool(name="ps", bufs=5, space="PSUM"))
        pst_p = ctx.enter_context(tc.tile_pool(name="pst", bufs=1, space="PSUM"))
        pso = ctx.enter_context(tc.tile_pool(name="pso", bufs=2, space="PSUM"))

        # ---------------- constants ----------------
        tri = const.tile([P, P], F32)
        nc.sync.dma_start(out=tri[:], in_=tri_in[:])
        eps_c = const.tile([P, 1], F32)
        nc.vector.memset(eps_c[:], EPS)
        ones_row = const.tile([1, P], F32)
        nc.vector.memset(ones_row[:], 1.0)
        ones_col = const.tile([P, 1], F32)
        nc.vector.memset(ones_col[:], 1.0)
        ident_bf = const.tile([P, P], BF16)
        make_identity(nc, ident_bf)

        # ---------------- load x into resident h tiles ----------------
        h = []
        for tb in range(TB):
            ht = hp.tile([P, D], F32, tag="h")
            nc.sync.dma_start(out=ht[:], in_=x_in[tb * P:(tb + 1) * P, :])
            h.append(ht)

        # =========== helpers ===========
        def cross_part_absmax(tiles, tag):
            """tiles: list of [128, n] f32 -> [128,1] all-partition absmax
            (batch-local global max), clamped at 1e-8."""
            acc = None
            for tl in tiles:
                c = colp.tile([P, 1], F32, tag="col", bufs=12)
                nc.vector.tensor_reduce(out=c[:], in_=tl[:], axis=AX.X,
                                        op=ALU.max, apply_absolute_value=True)
                if acc is None:
                    acc = c
                else:
                    nxt = colp.tile([P, 1], F32, tag="col", bufs=12)
                    nc.vector.tensor_max(out=nxt[:], in0=acc[:], in1=c[:])
                    acc = nxt
            m1 = stat.tile([1, 1], F32, tag="cpm1")
            nc.gpsimd.tensor_reduce(out=m1[:], in_=acc[:], axis=AX.C, op=ALU.max)
            mb = colp.tile([P, 1], F32, tag="mhat", bufs=4)
            nc.gpsimd.partition_broadcast(mb[:], m1[:], channels=P)
            mc = colp.tile([P, 1], F32, tag="mhat", bufs=4)
            nc.vector.tensor_scalar_max(out=mc[:], in0=mb[:], scalar1=1e-8)
            return mc

        def iscale_of(mhat, tag):
            # returns INVERSE scale 127/m (quantize multiplies by this)
            rcp = colp.tile([P, 1], F32, tag="isc", bufs=6)
            nc.vector.reciprocal(out=rcp[:], in_=mhat[:])
            inv = colp.tile([P, 1], F32, tag="isc", bufs=6)
            nc.vector.tensor_scalar_mul(out=inv[:], in0=rcp[:], scalar1=127.0)
            return inv

        def quantize_tiles(src_tiles, isc, n=D):
            """f32 [128,n] tiles -> bf16 [128,n] tiles (int8 round/saturate)."""
            res = []
            for st in src_tiles:
                q8 = qxp.tile([P, T], I8, tag="q8")
                nc.vector.tensor_scalar_mul(out=q8[:, :n], in0=st[:], scalar1=isc[:])
                qb = qtp.tile([P, T], BF16, tag="qt")
                nc.gpsimd.tensor_copy(out=qb[:, :n], in_=q8[:, :n])
                res.append(qb)
            return res

        def transpose_to(qtiles):
            """TB x [128, D] bf16 (rows=t) -> DB x [128, T] bf16 (rows=d)."""
            outt = [qtp.tile([P, T], BF16, tag="qt", name="qt") for _ in range(DB)]
            for tb in range(TB):
                for db in range(DB):
                    pst = pst_p.tile([P, P], BF16, tag="pstr", name="pstr")
                    nc.tensor.transpose(pst[:P, :P],
                                        qtiles[tb][:, db * P:(db + 1) * P],
                                        ident_bf[:])
                    nc.vector.tensor_copy(out=outt[db][:, tb * P:(tb + 1) * P],
                                          in_=pst[:P, :P])
            return outt

        def layer_norm(tiles):
            outs = []
            for tb in range(TB):
                st6 = stat.tile([P, 6], F32, tag="bn6")
                nc.vector.bn_stats(out=st6[:], in_=tiles[tb][:])
                mv = stat.tile([P, 2], F32, tag="bn2")
                nc.vector.bn_aggr(out=mv[:], in_=st6[:])
                std = colp.tile([P, 1], F32, tag="col", bufs=12)
                nc.scalar.activation(out=std[:], in_=mv[:, 1:2], func=ACTF.Sqrt,
                                     bias=eps_c[:], scale=1.0)
                rstd = colp.tile([P, 1], F32, tag="col", bufs=12)
                nc.vector.reciprocal(out=rstd[:], in_=std[:])
                ot = lnp.tile([P, D], F32, tag="ln")
                nc.vector.tensor_scalar(out=ot[:], in0=tiles[tb][:],
                                        scalar1=mv[:, 0:1], scalar2=rstd[:],
                                        op0=ALU.subtract, op1=ALU.mult)
                outs.append(ot)
            return outs

        # ---------------- ctx prep (encoder mean + quantize, once) ----------
        ctx_rows = []
        for b in range(B):
            pctx = ps.tile([P, VN], F32, tag="psum")
            for sb in range(S // P):
                et = lnp.tile([P, D], F32, tag="ln")
                nc.sync.dma_start(
                    out=et[:], in_=enc_in[b * S + sb * P: b * S + (sb + 1) * P, :])
                nc.tensor.matmul(pctx[:1, :D], ones_col[:], et[:],
                                 start=(sb == 0), stop=(sb == S // P - 1))
            cr = stat.tile([1, D], F32, tag="ctxr", bufs=B)
            nc.scalar.activation(out=cr[:], in_=pctx[:1, :D],
                                 func=ACTF.Copy, scale=1.0 / S)
            ctx_rows.append(cr)
        cacc = None
        for b in range(B):
            cm = stat.tile([1, 1], F32, tag="cm", bufs=4)
            nc.vector.tensor_reduce(out=cm[:], in_=ctx_rows[b][:], axis=AX.X,
                                    op=ALU.max, apply_absolute_value=True)
            if cacc is None:
                cacc = cm
            else:
                nx = stat.tile([1, 1], F32, tag="cm", bufs=4)
                nc.vector.tensor_max(out=nx[:], in0=cacc[:], in1=cm[:])
                cacc = nx
        cmb = colp.tile([P, 1], F32, tag="ctxm")
        nc.gpsimd.partition_broadcast(cmb[:], cacc[:], channels=P)
        cmc = colp.tile([P, 1], F32, tag="ctxm")
        nc.vector.tensor_scalar_max(out=cmc[:], in0=cmb[:], scalar1=1e-8)
        isc_ctx = iscale_of(cmc, "ctx")
        q8row = stat.tile([1, D], I8, tag="q8ctx")
        nc.vector.tensor_scalar_mul(out=q8row[:], in0=ctx_rows[0][:],
                                    scalar1=isc_ctx[:1, :])
        dctx8 = drp.tile([1, D], I8, tag="dctx8")
        nc.sync.dma_start(out=dctx8[:], in_=q8row[:])
        qctx8 = stat.tile([P, DB], I8, tag="qctx8")
        nc.sync.dma_start(out=qctx8[:],
                          in_=dctx8[:].rearrange("one (j p) -> p (one j)", p=P))
        qctxT = const.tile([P, DB], BF16)
        nc.scalar.copy(out=qctxT[:], in_=qctx8[:])

        # ---------------- transformer layers ----------------
        for l in range(L):
            wsq, wsk, wsv, wso, wse, ws1, ws2 = ws[l]

            wq_sb = [wsp.tile([P, D], BF16, tag="ws", name="ws") for _ in range(DB)]
            wk_sb = [wsp.tile([P, D], BF16, tag="ws", name="ws") for _ in range(DB)]
            wv_sb = [wsp.tile([P, D], BF16, tag="ws", name="ws") for _ in range(DB)]
            for db in range(DB):
                nc.sync.dma_start(out=wq_sb[db][:], in_=wq[l][db * P:(db + 1) * P, :])
                nc.sync.dma_start(out=wk_sb[db][:], in_=wk[l][db * P:(db + 1) * P, :])
                nc.sync.dma_start(out=wv_sb[db][:], in_=wv[l][db * P:(db + 1) * P, :])
            bq_sb = const.tile([P, DB], F32, tag="bqc")
            bk_sb = const.tile([P, DB], F32, tag="bkc")
            nc.sync.dma_start(out=bq_sb[:], in_=bq_t[l][:])
            nc.sync.dma_start(out=bk_sb[:], in_=bk_t[l][:])

            # --- ln1 + quantize + transpose ---
            ln1 = layer_norm(h)
            if dbg and l == 0:
                for tb in range(TB):
                    nc.sync.dma_start(out=dbg["dbg_ln1"][tb * P:(tb + 1) * P, :],
                                      in_=ln1[tb][:])
            m_ln1 = cross_part_absmax(ln1, f"ln1_{l}")
            isc1 = iscale_of(m_ln1, f"ln1_{l}")
            q1 = quantize_tiles(ln1, isc1)
            q1T = transpose_to(q1)

            # --- qkv matmuls ---
            sc_q = colp.tile([P, 1], F32, tag="sc", bufs=6)
            nc.scalar.mul(out=sc_q[:], in_=m_ln1[:], mul=wsq * INV_SQRT_HD / 127.0)
            sc_k = colp.tile([P, 1], F32, tag="sc", bufs=6)
            nc.scalar.mul(out=sc_k[:], in_=m_ln1[:], mul=wsk / 127.0)
            sc_v = colp.tile([P, 1], F32, tag="sc", bufs=6)
            nc.scalar.mul(out=sc_v[:], in_=m_ln1[:], mul=wsv / 127.0)
            qT, kT = [], []
            for ob in range(DB):
                pq = ps.tile([P, VN], F32, tag="psum")
                for db in range(DB):
                    nc.tensor.matmul(pq[:, :T], wq_sb[db][:, ob * P:(ob + 1) * P],
                                     q1T[db][:], start=(db == 0), stop=(db == DB - 1))
                qf = qkp.tile([P, T], F32, tag="qk")
                nc.scalar.activation(out=qf[:], in_=pq[:, :T], func=ACTF.Identity,
                                     bias=bq_sb[:, ob:ob + 1], scale=sc_q[:])
                qT.append(qf)
                pk = ps.tile([P, VN], F32, tag="psum")
                for db in range(DB):
                    nc.tensor.matmul(pk[:, :T], wk_sb[db][:, ob * P:(ob + 1) * P],
                                     q1T[db][:], start=(db == 0), stop=(db == DB - 1))
                kf = qkp.tile([P, T], F32, tag="qk")
                nc.scalar.activation(out=kf[:], in_=pk[:, :T], func=ACTF.Identity,
                                     bias=bk_sb[:, ob:ob + 1], scale=sc_k[:])
                kT.append(kf)
            if dbg and l == 0:
                for ob in range(DB):
                    nc.sync.dma_start(out=dbg["dbg_qT"][ob * P:(ob + 1) * P, :],
                                      in_=qT[ob][:])
            v_sb = []
            for tb in range(TB):
                pv = ps.tile([P, VN], F32, tag="psum")
                for db in range(DB):
                    nc.tensor.matmul(pv[:, :D], q1T[db][:, tb * P:(tb + 1) * P],
                                     wv_sb[db][:], start=(db == 0), stop=(db == DB - 1))
                vt = vp.tile([P, H, HD + 1], F32, tag="v")
                nc.scalar.activation(
                    out=vt[:, :, :HD],
                    in_=pv[:, :D].rearrange("p (h d) -> p h d", h=H),
                    func=ACTF.Identity, bias=0.0, scale=sc_v[:])
                nc.vector.memset(vt[:, :, HD:HD + 1], 1.0)
                v_sb.append(vt)

            # --- attention (fp32) ---
            attnT = [atp.tile([P, T], F32, tag="at", name="at") for _ in range(DB)]
            for hh in range(H):
                ob, off = hh // 2, (hh % 2) * HD
                po = pso.tile([P, VN], F32, tag="psumo")
                for kb in range(TB):
                    qoff = kb * P
                    pss = ps.tile([P, VN], F32, tag="psum")
                    nc.tensor.matmul(
                        pss[:, qoff:T],
                        kT[ob][off:off + HD, kb * P:(kb + 1) * P],
                        qT[ob][off:off + HD, qoff:T],
                        start=True, stop=True)
                    mrow = colp.tile([P, 1], F32, tag="mrow", bufs=6)
                    nc.vector.tensor_reduce(out=mrow[:], in_=pss[:, qoff:T],
                                            axis=AX.X, op=ALU.max)
                    nmrow = colp.tile([P, 1], F32, tag="mrow", bufs=6)
                    nc.scalar.mul(out=nmrow[:], in_=mrow[:], mul=-1.0)
                    emrow = colp.tile([P, 1], F32, tag="mrow", bufs=6)
                    nc.scalar.activation(out=emrow[:], in_=mrow[:], func=ACTF.Exp)
                    ex = expp.tile([P, T], F32, tag="exp")
                    if qoff:
                        nc.gpsimd.memset(ex[:, :qoff], 0.0)
                    nc.scalar.activation(out=ex[:, qoff:T], in_=pss[:, qoff:T],
                                         func=ACTF.Exp, bias=nmrow[:], scale=1.0)
                    nc.vector.tensor_mul(out=ex[:, qoff:qoff + P],
                                         in0=ex[:, qoff:qoff + P], in1=tri[:])
                    vh = vhp.tile([P, HD + 1], F32, tag="vh")
                    nc.vector.tensor_scalar_mul(
                        out=vh[:], in0=v_sb[kb][:, hh, :], scalar1=emrow[:])
                    nc.tensor.matmul(po[:HD + 1, qoff:T], vh[:], ex[:, qoff:T],
                                     start=(kb == 0), stop=(kb == TB - 1))
                rec = stat.tile([1, T], F32, tag="rec")
                nc.vector.reciprocal(out=rec[:], in_=po[HD:HD + 1, :T])
                pb = ps.tile([P, VN], F32, tag="psum")
                nc.tensor.matmul(pb[:HD, :T], ones_row[:1, :HD], rec[:],
                                 start=True, stop=True)
                nc.scalar.copy(out=attnT[ob][off:off + HD, :], in_=po[:HD, :T])
                nc.vector.tensor_mul(out=attnT[ob][off:off + HD, :],
                                     in0=attnT[ob][off:off + HD, :],
                                     in1=pb[:HD, :T])
            if dbg and l == 0:
                for ob in range(DB):
                    nc.sync.dma_start(out=dbg["dbg_attn"][ob * P:(ob + 1) * P, :],
                                      in_=attnT[ob][:])

            # --- attention output projection (Wo) + residual ---
            wo_sb = [wsp.tile([P, D], BF16, tag="ws", name="ws") for _ in range(DB)]
            we_sb = [wsp.tile([P, D], BF16, tag="ws", name="ws") for _ in range(DB)]
            for db in range(DB):
                nc.sync.dma_start(out=wo_sb[db][:], in_=wo[l][db * P:(db + 1) * P, :])
                nc.sync.dma_start(out=we_sb[db][:], in_=wenc[l][db * P:(db + 1) * P, :])
            m_o = cross_part_absmax(attnT, f"o_{l}")
            isc_o = iscale_of(m_o, f"o_{l}")
            qo = quantize_tiles(attnT, isc_o, n=T)
            sc_o = colp.tile([P, 1], F32, tag="sc", bufs=6)
            nc.scalar.mul(out=sc_o[:], in_=m_o[:], mul=wso / 127.0)
            for tb in range(TB):
                pw = ps.tile([P, VN], F32, tag="psum")
                for ob in range(DB):
                    nc.tensor.matmul(pw[:, :D], qo[ob][:, tb * P:(tb + 1) * P],
                                     wo_sb[ob][:], start=(ob == 0), stop=(ob == DB - 1))
                nc.vector.scalar_tensor_tensor(out=h[tb][:], in0=pw[:, :D],
                                               scalar=sc_o[:], in1=h[tb][:],
                                               op0=ALU.mult, op1=ALU.add)

            # --- encoder-context projection + residual (broadcast over t) ---
            pe = ps.tile([P, VN], F32, tag="psum")
            for db in range(DB):
                nc.tensor.matmul(pe[:1, :D], qctxT[:, db:db + 1], we_sb[db][:],
                                 start=(db == 0), stop=(db == DB - 1))
            enc_row = stat.tile([1, D], F32, tag="encrow", bufs=1)
            sc_e = stat.tile([1, 1], F32, tag="sc_e")
            nc.scalar.mul(out=sc_e[:], in_=cmc[:1, :], mul=wse / 127.0)
            nc.scalar.activation(out=enc_row[:], in_=pe[:1, :D],
                                 func=ACTF.Copy, scale=sc_e[:])
            pbe = ps.tile([P, VN], F32, tag="psum")
            nc.tensor.matmul(pbe[:, :D], ones_row[:1, :P], enc_row[:],
                             start=True, stop=True)
            for tb in range(TB):
                nc.vector.tensor_add(out=h[tb][:], in0=h[tb][:], in1=pbe[:, :D])

            # --- FFN ---
            ln3 = layer_norm(h)
            m_ln3 = cross_part_absmax(ln3, f"ln3_{l}")
            isc3 = iscale_of(m_ln3, f"ln3_{l}")
            q3 = quantize_tiles(ln3, isc3)
            q3T = transpose_to(q3)
            w1_sb = [w1p.tile([P, FF], BF16, tag="w1", name="w1") for _ in range(DB)]
            for db in range(DB):
                nc.sync.dma_start(out=w1_sb[db][:], in_=w1[l][db * P:(db + 1) * P, :])
            b1_sb = const.tile([P, FB], F32, tag="b1c")
            nc.sync.dma_start(out=b1_sb[:], in_=b1_t[l][:])
            sc_1 = colp.tile([P, 1], F32, tag="sc", bufs=6)
            nc.scalar.mul(out=sc_1[:], in_=m_ln3[:], mul=ws1 / 127.0)
            gel = []
            for fb in range(FB):
                pf = ps.tile([P, VN], F32, tag="psum")
                for db in range(DB):
                    nc.tensor.matmul(pf[:, :T], w1_sb[db][:, fb * P:(fb + 1) * P],
                                     q3T[db][:], start=(db == 0), stop=(db == DB - 1))
                gt = gelp.tile([P, T], F32, tag="gel")
                nc.scalar.activation(out=gt[:], in_=pf[:, :T], func=ACTF.Gelu,
                                     bias=b1_sb[:, fb:fb + 1], scale=sc_1[:])
                gel.append(gt)
            if dbg and l == 0:
                for fb in range(FB):
                    nc.sync.dma_start(out=dbg["dbg_gelu"][fb * P:(fb + 1) * P, :],
                                      in_=gel[fb][:])
            m_g = cross_part_absmax(gel, f"g_{l}")
            isc_g = iscale_of(m_g, f"g_{l}")
            sc_2 = colp.tile([P, 1], F32, tag="sc", bufs=6)
            nc.scalar.mul(out=sc_2[:], in_=m_g[:], mul=ws2 / 127.0)
            qg = []
            for fb in range(FB):
                g8 = qxp.tile([P, T], I8, tag="q8")
                nc.vector.tensor_scalar_mul(out=g8[:], in0=gel[fb][:], scalar1=isc_g[:])
                gq = qgp.tile([P, T], BF16, tag="qg")
                nc.gpsimd.tensor_copy(out=gq[:], in_=g8[:])
                qg.append(gq)
            w2_sb = [w2p.tile([P, D], BF16, tag="w2", name="w2") for _ in range(FB)]
            for fb in range(FB):
                nc.sync.dma_start(out=w2_sb[fb][:], in_=w2[l][fb * P:(fb + 1) * P, :])
            for tb in range(TB):
                pf2 = ps.tile([P, VN], F32, tag="psum")
                for fb in range(FB):
                    nc.tensor.matmul(pf2[:, :D], qg[fb][:, tb * P:(tb + 1) * P],
                                     w2_sb[fb][:], start=(fb == 0), stop=(fb == FB - 1))
                nc.vector.scalar_tensor_tensor(out=h[tb][:], in0=pf2[:, :D],
                                               scalar=sc_2[:], in1=h[tb][:],
                                               op0=ALU.mult, op1=ALU.add)
            if dbg:
                tgt = dbg["dbg_h0"] if l == 0 else dbg["dbg_h1"]
                for tb in range(TB):
                    nc.sync.dma_start(out=tgt[tb * P:(tb + 1) * P, :], in_=h[tb][:])

        # ---------------- final LN + output projection ----------------
        ws_out = ws[L]
        lnf = layer_norm(h)
        m_h = cross_part_absmax(lnf, "lnf")
        isc_h = iscale_of(m_h, "lnf")
        qh = quantize_tiles(lnf, isc_h)
        qhT = transpose_to(qh)
        sc_out = colp.tile([P, 1], F32, tag="sc_out")
        nc.scalar.mul(out=sc_out[:], in_=m_h[:], mul=ws_out / 127.0)
        nvb = VBF + (1 if VREM else 0)
        # process vocab blocks in pairs: one [128, 1024] bf16 staging tile per
        # (tb, pair) -> bigger, fewer output DMAs
        pair_starts = list(range(0, VBF - 1, 2))       # (0,1), (2,3), ... (60,61)
        tail = [VBF] if VREM else []                   # remainder block alone
        evict_i = 0
        for pv, v0 in enumerate(pair_starts + tail):
            pair = v0 < VBF - 1
            vws = [(v0, VN), (v0 + 1, VN)] if pair else [(v0, VREM)]
            w8 = [wop.tile([P, 2 * VN], I8, tag="w8", name="w8", bufs=5)
                  for _ in range(DB)]
            wtile = [wop.tile([P, 2 * VN], BF16, tag="wout", name="wout", bufs=5)
                     for _ in range(DB)]
            wn_tot = sum(vn for _, vn in vws)
            for db in range(DB):
                nc.sync.dma_start(out=w8[db][:, :wn_tot],
                                    in_=wout_in[db * P:(db + 1) * P,
                                                v0 * VN:v0 * VN + wn_tot])
                eng_up = nc.gpsimd if db % 2 == 0 else nc.vector
                eng_up.tensor_copy(out=wtile[db][:, :wn_tot],
                                   in_=w8[db][:, :wn_tot])
            for tb in range(TB):
                lt = outp.tile([P, 2 * VN], BF16, tag="lt")
                for si, (vb, vn) in enumerate(vws):
                    pl = ps.tile([P, VN], F32, tag="psum")
                    for db in range(DB):
                        nc.tensor.matmul(
                            pl[:, :vn],
                            qhT[db][:, tb * P:(tb + 1) * P],
                            wtile[db][:, si * VN:si * VN + vn],
                            start=(db == 0), stop=(db == DB - 1))
                    if evict_i % 2 == 0:
                        nc.scalar.activation(out=lt[:, si * VN:si * VN + vn],
                                             in_=pl[:, :vn], func=ACTF.Copy,
                                             scale=sc_out[:])
                    else:
                        nc.vector.tensor_scalar_mul(
                            out=lt[:, si * VN:si * VN + vn], in0=pl[:, :vn],
                            scalar1=sc_out[:])
                    evict_i += 1
                eng = nc.sync if (tb % 2 == 0) else nc.scalar
                eng.dma_start(
                    out=out[tb * P:(tb + 1) * P, v0 * VN:v0 * VN + wn_tot],
                    in_=lt[:, :wn_tot])


def _prep(inputs):
    """Host-side packing shared across cores; returns (common dict, wscales)."""
    f32 = np.float32
    ws = []
    common = {}
    for l in range(L):
        packed = []
        for name, W in [("wq", inputs["Wq"][l]), ("wk", inputs["Wk"][l]),
                        ("wv", inputs["Wv"][l]), ("wo", inputs["Wo"][l]),
                        ("wenc", inputs["Wenc"][l]), ("w1", inputs["W1"][l]),
                        ("w2", inputs["W2"][l])]:
            qWT, s = _quantize_weight(W)
            common[f"{name}{l}"] = qWT
            packed.append(s)
        ws.append(tuple(packed))
        common[f"bq{l}"] = _col_layout(
            np.asarray(inputs["bq"][l], f32) * f32(INV_SQRT_HD))
        common[f"bk{l}"] = _col_layout(inputs["bk"][l])
        common[f"b1{l}"] = _col_layout(inputs["b1"][l])
    qWoutT, s_out = _quantize_weight(inputs["Wout"])
    common["wout"] = np.ascontiguousarray(
        np.asarray(qWoutT, np.float32).astype(np.int8))
    ws.append(s_out)
    common["tri"] = np.triu(np.ones((P, P), dtype=f32))
    return common, ws


def kernel(**inputs):
    debug = bool(int(os.environ.get("BITGEN_DEBUG", "0")))
    common, ws = _prep(inputs)

    # Fast path assumes the model's declared fills: zero biases on the layers
    # without per-partition bias layout, identity LN affines.
    for nm in ["bo", "benc", "b2", "bout", "bv",
               "ln1b", "ln2b", "ln3b", "lnfb"]:
        assert not np.any(np.asarray(inputs[nm])), f"nonzero {nm} unsupported"
    for nm in ["ln1g", "ln2g", "ln3g", "lnfg"]:
        assert np.all(np.asarray(inputs[nm]) == 1.0), f"non-unit {nm} unsupported"

    key = ("v1", debug, tuple(np.asarray(w, np.float64).tobytes()
                              for w in (tuple(ws[l]) for l in range(L)))), ws[L]
    key = (repr(ws), debug)
    if key not in _NC_CACHE:
        _NC_CACHE[key] = build(ws, debug=debug)
    nc = _NC_CACHE[key]

    enc = np.asarray(inputs["encoder_output"], np.float32)
    x = np.asarray(inputs["x"], np.float32)
    in_maps = []
    for c in range(NCORES):
        m = dict(common)
        m["x"] = np.ascontiguousarray(x[c])
        # roll so this core's batch is the first S-row block (the kernel
        # quantizes ctx row 0 as its own batch, using all rows for the scale)
        m["enc"] = np.ascontiguousarray(
            np.roll(enc, -c, axis=0).reshape(B * S, D))
        in_maps.append(m)

    res = run_bass_kernel_spmd(nc, in_maps, list(range(NCORES)))
    outs = [res.results[c]["logits"].astype(np.float32) for c in range(NCORES)]
    return np.stack(outs, axis=0)


# revision 16
# speedup vs baseline: 1.2833x; 1.0052x over previous
"""Trainium2 Bass kernel for nn_BitGenModel (BitNet-style dense transformer).

Sharding: data-parallel over batch (B=8) across 8 NeuronCores; each core runs
the full 2-layer transformer + final 32000-vocab projection for its batch
element. Weights are pre-quantized (ternary, bf16) and pre-transposed on the
host; activation quantization (per-tensor absmax int8) runs on device with
batch-local scales.

Numerics:
- BitLinear integer matmuls run on the PE in bf16: |qx|<=127 and qW in
  {-1,0,1} are exactly representable and accumulation is fp32 in PSUM, so
  these matmuls are exact.
- Attention (scores, softmax, A@V) runs in fp32. Softmax subtracts a
  per-key-row max m_k and folds an exp(m_k) correction column into the V
  operand; the m_k cancel exactly in the normalization, so no cross-partition
  reductions are needed.
- LayerNorm uses DVE bn_stats/bn_aggr (fp32, free-dim reduction) in the
  natural [token, feature] layout.
"""
import math
import os

import numpy as np
import ml_dtypes

import concourse.bass as bass
import concourse.mybir as mybir
import concourse.tile as tile
from concourse import bacc
from concourse.bass_utils import run_bass_kernel_spmd
from concourse.masks import make_identity

F32 = mybir.dt.float32
BF16 = mybir.dt.bfloat16
I8 = mybir.dt.int8
AX = mybir.AxisListType
ALU = mybir.AluOpType
ACTF = mybir.ActivationFunctionType

B, T, S = 8, 512, 256
D, H, HD, FF, V, L = 512, 8, 64, 2048, 32000, 2
EPS = 1e-5
P = 128
TB = T // P            # 4 token blocks
DB = D // P            # 4 feature blocks
FB = FF // P           # 16 ffn blocks
VN = 512               # vocab tile width
VBF = V // VN          # 62 full vocab blocks
VREM = V - VBF * VN    # 256 remainder
NCORES = 8
INV_SQRT_HD = 1.0 / math.sqrt(HD)

_NC_CACHE = {}


def _quantize_weight(W):
    """Host-side eval-mode BitNet weight quantization (matches reference)."""
    W = np.asarray(W, dtype=np.float32)
    w_scale = np.maximum(np.abs(W).mean(dtype=np.float32), np.float32(1e-5))
    qW = np.sign(W) * (np.abs(W) > np.float32(0.5) * w_scale)
    qWT = np.ascontiguousarray(qW.astype(np.float32).T.astype(ml_dtypes.bfloat16))
    return qWT, float(w_scale)


def _col_layout(v):
    """[N] -> [128, N/128]: element [p, j] = v[j*128 + p] (per-partition cols)."""
    v = np.asarray(v, dtype=np.float32)
    n = v.shape[0]
    return np.ascontiguousarray(v.reshape(n // P, P).T)


def build(wscales, debug=False):
    nc = bacc.Bacc("TRN2", target_bir_lowering=False, debug=False,
                   num_devices=NCORES)

    x_in = nc.dram_tensor("x", [T, D], F32, kind="ExternalInput")
    enc_in = nc.dram_tensor("enc", [B * S, D], F32, kind="ExternalInput")
    tri_in = nc.dram_tensor("tri", [P, P], F32, kind="ExternalInput")

    wq, wk, wv, wo, wenc, w1, w2 = [], [], [], [], [], [], []
    bq, bk, b1 = [], [], []
    for l in range(L):
        wq.append(nc.dram_tensor(f"wq{l}", [D, D], BF16, kind="ExternalInput"))
        wk.append(nc.dram_tensor(f"wk{l}", [D, D], BF16, kind="ExternalInput"))
        wv.append(nc.dram_tensor(f"wv{l}", [D, D], BF16, kind="ExternalInput"))
        wo.append(nc.dram_tensor(f"wo{l}", [D, D], BF16, kind="ExternalInput"))
        wenc.append(nc.dram_tensor(f"wenc{l}", [D, D], BF16, kind="ExternalInput"))
        w1.append(nc.dram_tensor(f"w1{l}", [D, FF], BF16, kind="ExternalInput"))
        w2.append(nc.dram_tensor(f"w2{l}", [FF, D], BF16, kind="ExternalInput"))
        bq.append(nc.dram_tensor(f"bq{l}", [P, DB], F32, kind="ExternalInput"))
        bk.append(nc.dram_tensor(f"bk{l}", [P, DB], F32, kind="ExternalInput"))
        b1.append(nc.dram_tensor(f"b1{l}", [P, FB], F32, kind="ExternalInput"))
    wout_in = nc.dram_tensor("wout", [D, V], I8, kind="ExternalInput")

    out = nc.dram_tensor("logits", [T, V], BF16, kind="ExternalOutput")

    dbg = {}
    if debug:
        for name, shape in [("dbg_h0", [T, D]), ("dbg_h1", [T, D]),
                            ("dbg_ln1", [T, D]), ("dbg_attn", [D, T]),
                            ("dbg_qT", [D, T]), ("dbg_gelu", [FF, T])]:
            dbg[name] = nc.dram_tensor(name, shape, F32, kind="ExternalOutput")

    with tile.TileContext(nc) as tc:
        _body(nc, tc, wscales, x_in, enc_in, tri_in,
              wq, wk, wv, wo, wenc, w1, w2, bq, bk, b1, wout_in, out, dbg)
    nc.compile()
    return nc


def _body(nc, tc, ws, x_in, enc_in, tri_in,
          wq, wk, wv, wo, wenc, w1, w2, bq_t, bk_t, b1_t, wout_in, out, dbg):
    from contextlib import ExitStack
    ctx = ExitStack()
    with ctx:
        # ---------------- pools (uniform tile shape per pool) ----------------
        hp = ctx.enter_context(tc.tile_pool(name="hp", bufs=TB))
        const = ctx.enter_context(tc.tile_pool(name="const", bufs=1))
        lnp = ctx.enter_context(tc.tile_pool(name="lnp", bufs=TB + 1))
        stat = ctx.enter_context(tc.tile_pool(name="stat", bufs=2))
        colp = ctx.enter_context(tc.tile_pool(name="colp", bufs=2))
        qxp = ctx.enter_context(tc.tile_pool(name="qxp", bufs=3))
        qtp = ctx.enter_context(tc.tile_pool(name="qtp", bufs=9))
        wsp = ctx.enter_context(tc.tile_pool(name="wsp", bufs=12))
        w1p = ctx.enter_context(tc.tile_pool(name="w1p", bufs=DB))
        w2p = ctx.enter_context(tc.tile_pool(name="w2p", bufs=6))
        qkp = ctx.enter_context(tc.tile_pool(name="qkp", bufs=2 * DB))
        vp = ctx.enter_context(tc.tile_pool(name="vp", bufs=TB))
        vhp = ctx.enter_context(tc.tile_pool(name="vhp", bufs=3))
        expp = ctx.enter_context(tc.tile_pool(name="expp", bufs=4))
        atp = ctx.enter_context(tc.tile_pool(name="atp", bufs=TB))
        gelp = ctx.enter_context(tc.tile_pool(name="gelp", bufs=FB))
        qgp = ctx.enter_context(tc.tile_pool(name="qgp", bufs=FB))
        drp = ctx.enter_context(tc.tile_pool(name="drp", bufs=4, space="DRAM"))
        outp = ctx.enter_context(tc.tile_pool(name="outp", bufs=4))
        wop = ctx.enter_context(tc.tile_pool(name="wop", bufs=3))
        ps = ctx.enter_context(tc.tile_pool(name="ps", bufs=5, space="PSUM"))
        pst_p = ctx.enter_context(tc.tile_pool(name="pst", bufs=1, space="PSUM"))
        pso = ctx.enter_context(tc.tile_pool(name="pso", bufs=2, space="PSUM"))

        # ---------------- constants ----------------
        tri = const.tile([P, P], F32)
        nc.sync.dma_start(out=tri[:], in_=tri_in[:])
        eps_c = const.tile([P, 1], F32)
        nc.vector.memset(eps_c[:], EPS)
        ones_row = const.tile([1, P], F32)
        nc.vector.memset(ones_row[:], 1.0)
        ones_col = const.tile([P, 1], F32)
        nc.vector.memset(ones_col[:], 1.0)
        ident_bf = const.tile([P, P], BF16)
        make_identity(nc, ident_bf)

        # ---------------- load x into resident h tiles ----------------
        h = []
        for tb in range(TB):
            ht = hp.tile([P, D], F32, tag="h")
            nc.sync.dma_start(out=ht[:], in_=x_in[tb * P:(tb + 1) * P, :])
            h.append(ht)

        # =========== helpers ===========
        def cross_part_absmax(tiles, tag):
            """tiles: list of [128, n] f32 -> [128,1] all-partition absmax
            (batch-local global max), clamped at 1e-8."""
            acc = None
            for tl in tiles:
                c = colp.tile([P, 1], F32, tag="col", bufs=12)
                nc.vector.tensor_reduce(out=c[:], in_=tl[:], axis=AX.X,
                                        op=ALU.max, apply_absolute_value=True)
                if acc is None:
                    acc = c
                else:
                    nxt = colp.tile([P, 1], F32, tag="col", bufs=12)
                    nc.vector.tensor_max(out=nxt[:], in0=acc[:], in1=c[:])
                    acc = nxt
            m1 = stat.tile([1, 1], F32, tag="cpm1")
            nc.gpsimd.tensor_reduce(out=m1[:], in_=acc[:], axis=AX.C, op=ALU.max)
            mb = colp.tile([P, 1], F32, tag="mhat", bufs=4)
            nc.gpsimd.partition_broadcast(mb[:], m1[:], channels=P)
            mc = colp.tile([P, 1], F32, tag="mhat", bufs=4)
            nc.vector.tensor_scalar_max(out=mc[:], in0=mb[:], scalar1=1e-8)
            return mc

        def iscale_of(mhat, tag):
            # returns INVERSE scale 127/m (quantize multiplies by this)
            rcp = colp.tile([P, 1], F32, tag="isc", bufs=6)
            nc.vector.reciprocal(out=rcp[:], in_=mhat[:])
            inv = colp.tile([P, 1], F32, tag="isc", bufs=6)
            nc.vector.tensor_scalar_mul(out=inv[:], in0=rcp[:], scalar1=127.0)
            return inv

        def quantize_tiles(src_tiles, isc, n=D):
            """f32 [128,n] tiles -> bf16 [128,n] tiles (int8 round/saturate)."""
            res = []
            for st in src_tiles:
                q8 = qxp.tile([P, T], I8, tag="q8")
                nc.vector.tensor_scalar_mul(out=q8[:, :n], in0=st[:], scalar1=isc[:])
                qb = qtp.tile([P, T], BF16, tag="qt")
                nc.gpsimd.tensor_copy(out=qb[:, :n], in_=q8[:, :n])
                res.append(qb)
            return res

        def transpose_to(qtiles):
            """TB x [128, D] bf16 (rows=t) -> DB x [128, T] bf16 (rows=d)."""
            outt = [qtp.tile([P, T], BF16, tag="qt", name="qt") for _ in range(DB)]
            for tb in range(TB):
                for db in range(DB):
                    pst = pst_p.tile([P, P], BF16, tag="pstr", name="pstr")
                    nc.tensor.transpose(pst[:P, :P],
                                        qtiles[tb][:, db * P:(db + 1) * P],
                                        ident_bf[:])
                    nc.vector.tensor_copy(out=outt[db][:, tb * P:(tb + 1) * P],
                                          in_=pst[:P, :P])
            return outt

        def layer_norm(tiles):
            outs = []
            for tb in range(TB):
                st6 = stat.tile([P, 6], F32, tag="bn6")
                nc.vector.bn_stats(out=st6[:], in_=tiles[tb][:])
                mv = stat.tile([P, 2], F32, tag="bn2")
                nc.vector.bn_aggr(out=mv[:], in_=st6[:])
                std = colp.tile([P, 1], F32, tag="col", bufs=12)
                nc.scalar.activation(out=std[:], in_=mv[:, 1:2], func=ACTF.Sqrt,
                                     bias=eps_c[:], scale=1.0)
                rstd = colp.tile([P, 1], F32, tag="col", bufs=12)
                nc.vector.reciprocal(out=rstd[:], in_=std[:])
                ot = lnp.tile([P, D], F32, tag="ln")
                nc.vector.tensor_scalar(out=ot[:], in0=tiles[tb][:],
                                        scalar1=mv[:, 0:1], scalar2=rstd[:],
                                        op0=ALU.subtract, op1=ALU.mult)
                outs.append(ot)
            return outs

        # ---------------- ctx prep (encoder mean + quantize, once) ----------
        ctx_rows = []
        for b in range(B):
            pctx = ps.tile([P, VN], F32, tag="psum")
            for sb in range(S // P):
                et = lnp.tile([P, D], F32, tag="ln")
                nc.sync.dma_start(
                    out=et[:], in_=enc_in[b * S + sb * P: b * S + (sb + 1) * P, :])
                nc.tensor.matmul(pctx[:1, :D], ones_col[:], et[:],
                                 start=(sb == 0), stop=(sb == S // P - 1))
            cr = stat.tile([1, D], F32, tag="ctxr", bufs=B)
            nc.scalar.activation(out=cr[:], in_=pctx[:1, :D],
                                 func=ACTF.Copy, scale=1.0 / S)
            ctx_rows.append(cr)
        cacc = None
        for b in range(B):
            cm = stat.tile([1, 1], F32, tag="cm", bufs=4)
            nc.vector.tensor_reduce(out=cm[:], in_=ctx_rows[b][:], axis=AX.X,
                                    op=ALU.max, apply_absolute_value=True)
            if cacc is None:
                cacc = cm
            else:
                nx = stat.tile([1, 1], F32, tag="cm", bufs=4)
                nc.vector.tensor_max(out=nx[:], in0=cacc[:], in1=cm[:])
                cacc = nx
        cmb = colp.tile([P, 1], F32, tag="ctxm")
        nc.gpsimd.partition_broadcast(cmb[:], cacc[:], channels=P)
        cmc = colp.tile([P, 1], F32, tag="ctxm")
        nc.vector.tensor_scalar_max(out=cmc[:], in0=cmb[:], scalar1=1e-8)
        isc_ctx = iscale_of(cmc, "ctx")
        q8row = stat.tile([1, D], I8, tag="q8ctx")
        nc.vector.tensor_scalar_mul(out=q8row[:], in0=ctx_rows[0][:],
                                    scalar1=isc_ctx[:1, :])
        dctx8 = drp.tile([1, D], I8, tag="dctx8")
        nc.sync.dma_start(out=dctx8[:], in_=q8row[:])
        qctx8 = stat.tile([P, DB], I8, tag="qctx8")
        nc.sync.dma_start(out=qctx8[:],
                          in_=dctx8[:].rearrange("one (j p) -> p (one j)", p=P))
        qctxT = const.tile([P, DB], BF16)
        nc.scalar.copy(out=qctxT[:], in_=qctx8[:])

        # ---------------- transformer layers ----------------
        for l in range(L):
            wsq, wsk, wsv, wso, wse, ws1, ws2 = ws[l]

            wq_sb = [wsp.tile([P, D], BF16, tag="ws", name="ws") for _ in range(DB)]
            wk_sb = [wsp.tile([P, D], BF16, tag="ws", name="ws") for _ in range(DB)]
            wv_sb = [wsp.tile([P, D], BF16, tag="ws", name="ws") for _ in range(DB)]
            for db in range(DB):
                nc.sync.dma_start(out=wq_sb[db][:], in_=wq[l][db * P:(db + 1) * P, :])
                nc.sync.dma_start(out=wk_sb[db][:], in_=wk[l][db * P:(db + 1) * P, :])
                nc.sync.dma_start(out=wv_sb[db][:], in_=wv[l][db * P:(db + 1) * P, :])
            w1_sb = [w1p.tile([P, FF], BF16, tag="w1", name="w1") for _ in range(DB)]
            for db in range(DB):
                nc.sync.dma_start(out=w1_sb[db][:], in_=w1[l][db * P:(db + 1) * P, :])
            bq_sb = const.tile([P, DB], F32, tag="bqc")
            bk_sb = const.tile([P, DB], F32, tag="bkc")
            nc.sync.dma_start(out=bq_sb[:], in_=bq_t[l][:])
            nc.sync.dma_start(out=bk_sb[:], in_=bk_t[l][:])

            # --- ln1 + quantize + transpose ---
            ln1 = layer_norm(h)
            if dbg and l == 0:
                for tb in range(TB):
                    nc.sync.dma_start(out=dbg["dbg_ln1"][tb * P:(tb + 1) * P, :],
                                      in_=ln1[tb][:])
            m_ln1 = cross_part_absmax(ln1, f"ln1_{l}")
            isc1 = iscale_of(m_ln1, f"ln1_{l}")
            q1 = quantize_tiles(ln1, isc1)
            q1T = transpose_to(q1)

            # --- qkv matmuls ---
            sc_q = colp.tile([P, 1], F32, tag="sc", bufs=6)
            nc.scalar.mul(out=sc_q[:], in_=m_ln1[:], mul=wsq * INV_SQRT_HD / 127.0)
            sc_k = colp.tile([P, 1], F32, tag="sc", bufs=6)
            nc.scalar.mul(out=sc_k[:], in_=m_ln1[:], mul=wsk / 127.0)
            sc_v = colp.tile([P, 1], F32, tag="sc", bufs=6)
            nc.scalar.mul(out=sc_v[:], in_=m_ln1[:], mul=wsv / 127.0)
            qT, kT = [], []
            for ob in range(DB):
                pq = ps.tile([P, VN], F32, tag="psum")
                for db in range(DB):
                    nc.tensor.matmul(pq[:, :T], wq_sb[db][:, ob * P:(ob + 1) * P],
                                     q1T[db][:], start=(db == 0), stop=(db == DB - 1))
                qf = qkp.tile([P, T], F32, tag="qk")
                nc.scalar.activation(out=qf[:], in_=pq[:, :T], func=ACTF.Identity,
                                     bias=bq_sb[:, ob:ob + 1], scale=sc_q[:])
                qT.append(qf)
                pk = ps.tile([P, VN], F32, tag="psum")
                for db in range(DB):
                    nc.tensor.matmul(pk[:, :T], wk_sb[db][:, ob * P:(ob + 1) * P],
                                     q1T[db][:], start=(db == 0), stop=(db == DB - 1))
                kf = qkp.tile([P, T], F32, tag="qk")
                nc.scalar.activation(out=kf[:], in_=pk[:, :T], func=ACTF.Identity,
                                     bias=bk_sb[:, ob:ob + 1], scale=sc_k[:])
                kT.append(kf)
            if dbg and l == 0:
                for ob in range(DB):
                    nc.sync.dma_start(out=dbg["dbg_qT"][ob * P:(ob + 1) * P, :],
                                      in_=qT[ob][:])
            v_sb = []
            for tb in range(TB):
                pv = ps.tile([P, VN], F32, tag="psum")
                for db in range(DB):
                    nc.tensor.matmul(pv[:, :D], q1T[db][:, tb * P:(tb + 1) * P],
                                     wv_sb[db][:], start=(db == 0), stop=(db == DB - 1))
                vt = vp.tile([P, H, HD + 1], F32, tag="v")
                nc.scalar.activation(
                    out=vt[:, :, :HD],
                    in_=pv[:, :D].rearrange("p (h d) -> p h d", h=H),
                    func=ACTF.Identity, bias=0.0, scale=sc_v[:])
                nc.vector.memset(vt[:, :, HD:HD + 1], 1.0)
                v_sb.append(vt)

            # --- attention (fp32) ---
            attnT = [atp.tile([P, T], F32, tag="at", name="at") for _ in range(DB)]
            for hh in range(H):
                ob, off = hh // 2, (hh % 2) * HD
                po = pso.tile([P, VN], F32, tag="psumo")
                for kb in range(TB):
                    qoff = kb * P
                    pss = ps.tile([P, VN], F32, tag="psum")
                    nc.tensor.matmul(
                        pss[:, qoff:T],
                        kT[ob][off:off + HD, kb * P:(kb + 1) * P],
                        qT[ob][off:off + HD, qoff:T],
                        start=True, stop=True)
                    mrow = colp.tile([P, 1], F32, tag="mrow", bufs=6)
                    nc.vector.tensor_reduce(out=mrow[:], in_=pss[:, qoff:T],
                                            axis=AX.X, op=ALU.max)
                    nmrow = colp.tile([P, 1], F32, tag="mrow", bufs=6)
                    nc.scalar.mul(out=nmrow[:], in_=mrow[:], mul=-1.0)
                    emrow = colp.tile([P, 1], F32, tag="mrow", bufs=6)
                    nc.scalar.activation(out=emrow[:], in_=mrow[:], func=ACTF.Exp)
                    ex = expp.tile([P, T], F32, tag="exp")
                    if qoff:
                        nc.gpsimd.memset(ex[:, :qoff], 0.0)
                    nc.scalar.activation(out=ex[:, qoff:T], in_=pss[:, qoff:T],
                                         func=ACTF.Exp, bias=nmrow[:], scale=1.0)
                    nc.vector.tensor_mul(out=ex[:, qoff:qoff + P],
                                         in0=ex[:, qoff:qoff + P], in1=tri[:])
                    vh = vhp.tile([P, HD + 1], F32, tag="vh")
                    nc.vector.tensor_scalar_mul(
                        out=vh[:], in0=v_sb[kb][:, hh, :], scalar1=emrow[:])
                    nc.tensor.matmul(po[:HD + 1, qoff:T], vh[:], ex[:, qoff:T],
                                     start=(kb == 0), stop=(kb == TB - 1))
                rec = stat.tile([1, T], F32, tag="rec")
                nc.vector.reciprocal(out=rec[:], in_=po[HD:HD + 1, :T])
                pb = ps.tile([P, VN], F32, tag="psum")
                nc.tensor.matmul(pb[:HD, :T], ones_row[:1, :HD], rec[:],
                                 start=True, stop=True)
                nc.scalar.copy(out=attnT[ob][off:off + HD, :], in_=po[:HD, :T])
                nc.vector.tensor_mul(out=attnT[ob][off:off + HD, :],
                                     in0=attnT[ob][off:off + HD, :],
                                     in1=pb[:HD, :T])
            if dbg and l == 0:
                for ob in range(DB):
                    nc.sync.dma_start(out=dbg["dbg_attn"][ob * P:(ob + 1) * P, :],
                                      in_=attnT[ob][:])

            # --- attention output projection (Wo) + residual ---
            wo_sb = [wsp.tile([P, D], BF16, tag="ws", name="ws") for _ in range(DB)]
            we_sb = [wsp.tile([P, D], BF16, tag="ws", name="ws") for _ in range(DB)]
            for db in range(DB):
                nc.sync.dma_start(out=wo_sb[db][:], in_=wo[l][db * P:(db + 1) * P, :])
                nc.sync.dma_start(out=we_sb[db][:], in_=wenc[l][db * P:(db + 1) * P, :])
            m_o = cross_part_absmax(attnT, f"o_{l}")
            isc_o = iscale_of(m_o, f"o_{l}")
            qo = quantize_tiles(attnT, isc_o, n=T)
            sc_o = colp.tile([P, 1], F32, tag="sc", bufs=6)
            nc.scalar.mul(out=sc_o[:], in_=m_o[:], mul=wso / 127.0)
            for tb in range(TB):
                pw = ps.tile([P, VN], F32, tag="psum")
                for ob in range(DB):
                    nc.tensor.matmul(pw[:, :D], qo[ob][:, tb * P:(tb + 1) * P],
                                     wo_sb[ob][:], start=(ob == 0), stop=(ob == DB - 1))
                nc.vector.scalar_tensor_tensor(out=h[tb][:], in0=pw[:, :D],
                                               scalar=sc_o[:], in1=h[tb][:],
                                               op0=ALU.mult, op1=ALU.add)

            # --- encoder-context projection + residual (broadcast over t) ---
            pe = ps.tile([P, VN], F32, tag="psum")
            for db in range(DB):
                nc.tensor.matmul(pe[:1, :D], qctxT[:, db:db + 1], we_sb[db][:],
                                 start=(db == 0), stop=(db == DB - 1))
            enc_row = stat.tile([1, D], F32, tag="encrow", bufs=1)
            sc_e = stat.tile([1, 1], F32, tag="sc_e")
            nc.scalar.mul(out=sc_e[:], in_=cmc[:1, :], mul=wse / 127.0)
            nc.scalar.activation(out=enc_row[:], in_=pe[:1, :D],
                                 func=ACTF.Copy, scale=sc_e[:])
            pbe = ps.tile([P, VN], F32, tag="psum")
            nc.tensor.matmul(pbe[:, :D], ones_row[:1, :P], enc_row[:],
                             start=True, stop=True)
            for tb in range(TB):
                nc.vector.tensor_add(out=h[tb][:], in0=h[tb][:], in1=pbe[:, :D])

            # --- FFN ---
            ln3 = layer_norm(h)
            m_ln3 = cross_part_absmax(ln3, f"ln3_{l}")
            isc3 = iscale_of(m_ln3, f"ln3_{l}")
            q3 = quantize_tiles(ln3, isc3)
            q3T = transpose_to(q3)
            b1_sb = const.tile([P, FB], F32, tag="b1c")
            nc.sync.dma_start(out=b1_sb[:], in_=b1_t[l][:])
            sc_1 = colp.tile([P, 1], F32, tag="sc", bufs=6)
            nc.scalar.mul(out=sc_1[:], in_=m_ln3[:], mul=ws1 / 127.0)
            gel = []
            for fb in range(FB):
                pf = ps.tile([P, VN], F32, tag="psum")
                for db in range(DB):
                    nc.tensor.matmul(pf[:, :T], w1_sb[db][:, fb * P:(fb + 1) * P],
                                     q3T[db][:], start=(db == 0), stop=(db == DB - 1))
                gt = gelp.tile([P, T], F32, tag="gel")
                nc.scalar.activation(out=gt[:], in_=pf[:, :T], func=ACTF.Gelu,
                                     bias=b1_sb[:, fb:fb + 1], scale=sc_1[:])
                gel.append(gt)
            if dbg and l == 0:
                for fb in range(FB):
                    nc.sync.dma_start(out=dbg["dbg_gelu"][fb * P:(fb + 1) * P, :],
                                      in_=gel[fb][:])
            m_g = cross_part_absmax(gel, f"g_{l}")
            isc_g = iscale_of(m_g, f"g_{l}")
            sc_2 = colp.tile([P, 1], F32, tag="sc", bufs=6)
            nc.scalar.mul(out=sc_2[:], in_=m_g[:], mul=ws2 / 127.0)
            qg = []
            for fb in range(FB):
                g8 = qxp.tile([P, T], I8, tag="q8")
                nc.vector.tensor_scalar_mul(out=g8[:], in0=gel[fb][:], scalar1=isc_g[:])
                gq = qgp.tile([P, T], BF16, tag="qg")
                nc.gpsimd.tensor_copy(out=gq[:], in_=g8[:])
                qg.append(gq)
            pf2 = [ps.tile([P, VN], F32, tag="psum", name="pf2") for _ in range(TB)]
            for fb in range(FB):
                w2t = w2p.tile([P, D], BF16, tag="w2", name="w2", bufs=6)
                nc.sync.dma_start(out=w2t[:], in_=w2[l][fb * P:(fb + 1) * P, :])
                for tb in range(TB):
                    nc.tensor.matmul(pf2[tb][:, :D], qg[fb][:, tb * P:(tb + 1) * P],
                                     w2t[:], start=(fb == 0), stop=(fb == FB - 1))
            for tb in range(TB):
                nc.vector.scalar_tensor_tensor(out=h[tb][:], in0=pf2[tb][:, :D],
                                               scalar=sc_2[:], in1=h[tb][:],
                                               op0=ALU.mult, op1=ALU.add)
            if dbg:
                tgt = dbg["dbg_h0"] if l == 0 else dbg["dbg_h1"]
                for tb in range(TB):
                    nc.sync.dma_start(out=tgt[tb * P:(tb + 1) * P, :], in_=h[tb][:])

        # ---------------- final LN + output projection ----------------
        ws_out = ws[L]
        lnf = layer_norm(h)
        m_h = cross_part_absmax(lnf, "lnf")
        isc_h = iscale_of(m_h, "lnf")
        qh = quantize_tiles(lnf, isc_h)
        qhT = transpose_to(qh)
        sc_out = colp.tile([P, 1], F32, tag="sc_out")
        nc.scalar.mul(out=sc_out[:], in_=m_h[:], mul=ws_out / 127.0)
        nvb = VBF + (1 if VREM else 0)
        # process vocab blocks in pairs: one [128, 1024] bf16 staging tile per
        # (tb, pair) -> bigger, fewer output DMAs
        pair_starts = list(range(0, VBF - 1, 2))       # (0,1), (2,3), ... (60,61)
        tail = [VBF] if VREM else []                   # remainder block alone
        evict_i = 0
        for pv, v0 in enumerate(pair_starts + tail):
            pair = v0 < VBF - 1
            vws = [(v0, VN), (v0 + 1, VN)] if pair else [(v0, VREM)]
            w8 = [wop.tile([P, 2 * VN], I8, tag="w8", name="w8", bufs=5)
                  for _ in range(DB)]
            wtile = [wop.tile([P, 2 * VN], BF16, tag="wout", name="wout", bufs=5)
                     for _ in range(DB)]
            wn_tot = sum(vn for _, vn in vws)
            for db in range(DB):
                nc.sync.dma_start(out=w8[db][:, :wn_tot],
                                    in_=wout_in[db * P:(db + 1) * P,
                                                v0 * VN:v0 * VN + wn_tot])
                eng_up = nc.gpsimd if db % 2 == 0 else nc.vector
                eng_up.tensor_copy(out=wtile[db][:, :wn_tot],
                                   in_=w8[db][:, :wn_tot])
            for tb in range(TB):
                lt = outp.tile([P, 2 * VN], BF16, tag="lt")
                for si, (vb, vn) in enumerate(vws):
                    pl = ps.tile([P, VN], F32, tag="psum")
                    for db in range(DB):
                        nc.tensor.matmul(
                            pl[:, :vn],
                            qhT[db][:, tb * P:(tb + 1) * P],
                            wtile[db][:, si * VN:si * VN + vn],
                            start=(db == 0), stop=(db == DB - 1))
                    if evict_i % 2 == 0:
                        nc.scalar.activation(out=lt[:, si * VN:si * VN + vn],
                                             in_=pl[:, :vn], func=ACTF.Copy,
                                             scale=sc_out[:])
                    else:
                        nc.vector.tensor_scalar_mul(
                            out=lt[:, si * VN:si * VN + vn], in0=pl[:, :vn],
                            scalar1=sc_out[:])
                    evict_i += 1
                eng = nc.sync if (tb % 2 == 0) else nc.scalar
                eng.dma_start(
                    out=out[tb * P:(tb + 1) * P, v0 * VN:v0 * VN + wn_tot],
                    in_=lt[:, :wn_tot])


def _prep(inputs):
    """Host-side packing shared across cores; returns (common dict, wscales)."""
    f32 = np.float32
    ws = []
    common = {}
    for l in range(L):
        packed = []
        for name, W in [("wq", inputs["Wq"][l]), ("wk", inputs["Wk"][l]),
                        ("wv", inputs["Wv"][l]), ("wo", inputs["Wo"][l]),
                        ("wenc", inputs["Wenc"][l]), ("w1", inputs["W1"][l]),
                        ("w2", inputs["W2"][l])]:
            qWT, s = _quantize_weight(W)
            common[f"{name}{l}"] = qWT
            packed.append(s)
        ws.append(tuple(packed))
        common[f"bq{l}"] = _col_layout(
            np.asarray(inputs["bq"][l], f32) * f32(INV_SQRT_HD))
        common[f"bk{l}"] = _col_layout(inputs["bk"][l])
        common[f"b1{l}"] = _col_layout(inputs["b1"][l])
    qWoutT, s_out = _quantize_weight(inputs["Wout"])
    common["wout"] = np.ascontiguousarray(
        np.asarray(qWoutT, np.float32).astype(np.int8))
    ws.append(s_out)
    common["tri"] = np.triu(np.ones((P, P), dtype=f32))
    return common, ws


def kernel(**inputs):
    debug = bool(int(os.environ.get("BITGEN_DEBUG", "0")))
    common, ws = _prep(inputs)

    # Fast path assumes the model's declared fills: zero biases on the layers
    # without per-partition bias layout, identity LN affines.
    for nm in ["bo", "benc", "b2", "bout", "bv",
               "ln1b", "ln2b", "ln3b", "lnfb"]:
        assert not np.any(np.asarray(inputs[nm])), f"nonzero {nm} unsupported"
    for nm in ["ln1g", "ln2g", "ln3g", "lnfg"]:
        assert np.all(np.asarray(inputs[nm]) == 1.0), f"non-unit {nm} unsupported"

    key = ("v1", debug, tuple(np.asarray(w, np.float64).tobytes()
                              for w in (tuple(ws[l]) for l in range(L)))), ws[L]
    key = (repr(ws), debug)
    if key not in _NC_CACHE:
        _NC_CACHE[key] = build(ws, debug=debug)
    nc = _NC_CACHE[key]

    enc = np.asarray(inputs["encoder_output"], np.float32)
    x = np.asarray(inputs["x"], np.float32)
    in_maps = []
    for c in range(NCORES):
        m = dict(common)
        m["x"] = np.ascontiguousarray(x[c])
        # roll so this core's batch is the first S-row block (the kernel
        # quantizes ctx row 0 as its own batch, using all rows for the scale)
        m["enc"] = np.ascontiguousarray(
            np.roll(enc, -c, axis=0).reshape(B * S, D))
        in_maps.append(m)

    res = run_bass_kernel_spmd(nc, in_maps, list(range(NCORES)))
    outs = [res.results[c]["logits"].astype(np.float32) for c in range(NCORES)]
    return np.stack(outs, axis=0)


# revision 17
# speedup vs baseline: 1.2941x; 1.0084x over previous
"""Trainium2 Bass kernel for nn_BitGenModel (BitNet-style dense transformer).

Sharding: data-parallel over batch (B=8) across 8 NeuronCores; each core runs
the full 2-layer transformer + final 32000-vocab projection for its batch
element. Weights are pre-quantized (ternary, bf16) and pre-transposed on the
host; activation quantization (per-tensor absmax int8) runs on device with
batch-local scales.

Numerics:
- BitLinear integer matmuls run on the PE in bf16: |qx|<=127 and qW in
  {-1,0,1} are exactly representable and accumulation is fp32 in PSUM, so
  these matmuls are exact.
- Attention (scores, softmax, A@V) runs in fp32. Softmax subtracts a
  per-key-row max m_k and folds an exp(m_k) correction column into the V
  operand; the m_k cancel exactly in the normalization, so no cross-partition
  reductions are needed.
- LayerNorm uses DVE bn_stats/bn_aggr (fp32, free-dim reduction) in the
  natural [token, feature] layout.
"""
import math
import os

import numpy as np
import ml_dtypes

import concourse.bass as bass
import concourse.mybir as mybir
import concourse.tile as tile
from concourse import bacc
from concourse.bass_utils import run_bass_kernel_spmd
from concourse.masks import make_identity

F32 = mybir.dt.float32
BF16 = mybir.dt.bfloat16
I8 = mybir.dt.int8
AX = mybir.AxisListType
ALU = mybir.AluOpType
ACTF = mybir.ActivationFunctionType

B, T, S = 8, 512, 256
D, H, HD, FF, V, L = 512, 8, 64, 2048, 32000, 2
EPS = 1e-5
P = 128
TB = T // P            # 4 token blocks
DB = D // P            # 4 feature blocks
FB = FF // P           # 16 ffn blocks
VN = 512               # vocab tile width
VBF = V // VN          # 62 full vocab blocks
VREM = V - VBF * VN    # 256 remainder
NCORES = 8
INV_SQRT_HD = 1.0 / math.sqrt(HD)

_NC_CACHE = {}


def _quantize_weight(W):
    """Host-side eval-mode BitNet weight quantization (matches reference)."""
    W = np.asarray(W, dtype=np.float32)
    w_scale = np.maximum(np.abs(W).mean(dtype=np.float32), np.float32(1e-5))
    qW = np.sign(W) * (np.abs(W) > np.float32(0.5) * w_scale)
    qWT = np.ascontiguousarray(qW.astype(np.float32).T.astype(ml_dtypes.bfloat16))
    return qWT, float(w_scale)


def _col_layout(v):
    """[N] -> [128, N/128]: element [p, j] = v[j*128 + p] (per-partition cols)."""
    v = np.asarray(v, dtype=np.float32)
    n = v.shape[0]
    return np.ascontiguousarray(v.reshape(n // P, P).T)


def build(wscales, debug=False):
    nc = bacc.Bacc("TRN2", target_bir_lowering=False, debug=False,
                   num_devices=NCORES)

    x_in = nc.dram_tensor("x", [T, D], F32, kind="ExternalInput")
    enc_in = nc.dram_tensor("enc", [B * S, D], F32, kind="ExternalInput")
    tri_in = nc.dram_tensor("tri", [P, P], F32, kind="ExternalInput")

    wq, wk, wv, wo, wenc, w1, w2 = [], [], [], [], [], [], []
    bq, bk, b1 = [], [], []
    for l in range(L):
        wq.append(nc.dram_tensor(f"wq{l}", [D, D], BF16, kind="ExternalInput"))
        wk.append(nc.dram_tensor(f"wk{l}", [D, D], BF16, kind="ExternalInput"))
        wv.append(nc.dram_tensor(f"wv{l}", [D, D], BF16, kind="ExternalInput"))
        wo.append(nc.dram_tensor(f"wo{l}", [D, D], BF16, kind="ExternalInput"))
        wenc.append(nc.dram_tensor(f"wenc{l}", [D, D], BF16, kind="ExternalInput"))
        w1.append(nc.dram_tensor(f"w1{l}", [D, FF], BF16, kind="ExternalInput"))
        w2.append(nc.dram_tensor(f"w2{l}", [FF, D], BF16, kind="ExternalInput"))
        bq.append(nc.dram_tensor(f"bq{l}", [P, DB], F32, kind="ExternalInput"))
        bk.append(nc.dram_tensor(f"bk{l}", [P, DB], F32, kind="ExternalInput"))
        b1.append(nc.dram_tensor(f"b1{l}", [P, FB], F32, kind="ExternalInput"))
    wout_in = nc.dram_tensor("wout", [D, V], I8, kind="ExternalInput")

    out = nc.dram_tensor("logits", [T, V], BF16, kind="ExternalOutput")

    dbg = {}
    if debug:
        for name, shape in [("dbg_h0", [T, D]), ("dbg_h1", [T, D]),
                            ("dbg_ln1", [T, D]), ("dbg_attn", [D, T]),
                            ("dbg_qT", [D, T]), ("dbg_gelu", [FF, T])]:
            dbg[name] = nc.dram_tensor(name, shape, F32, kind="ExternalOutput")

    with tile.TileContext(nc) as tc:
        _body(nc, tc, wscales, x_in, enc_in, tri_in,
              wq, wk, wv, wo, wenc, w1, w2, bq, bk, b1, wout_in, out, dbg)
    nc.compile()
    return nc


def _body(nc, tc, ws, x_in, enc_in, tri_in,
          wq, wk, wv, wo, wenc, w1, w2, bq_t, bk_t, b1_t, wout_in, out, dbg):
    from contextlib import ExitStack
    ctx = ExitStack()
    with ctx:
        # ---------------- pools (uniform tile shape per pool) ----------------
        hp = ctx.enter_context(tc.tile_pool(name="hp", bufs=TB))
        const = ctx.enter_context(tc.tile_pool(name="const", bufs=1))
        lnp = ctx.enter_context(tc.tile_pool(name="lnp", bufs=TB + 1))
        stat = ctx.enter_context(tc.tile_pool(name="stat", bufs=2))
        colp = ctx.enter_context(tc.tile_pool(name="colp", bufs=2))
        qxp = ctx.enter_context(tc.tile_pool(name="qxp", bufs=3))
        qtp = ctx.enter_context(tc.tile_pool(name="qtp", bufs=9))
        wsp = ctx.enter_context(tc.tile_pool(name="wsp", bufs=12))
        w1p = ctx.enter_context(tc.tile_pool(name="w1p", bufs=DB))
        w2p = ctx.enter_context(tc.tile_pool(name="w2p", bufs=6))
        qkp = ctx.enter_context(tc.tile_pool(name="qkp", bufs=2 * DB))
        vp = ctx.enter_context(tc.tile_pool(name="vp", bufs=TB))
        vhp = ctx.enter_context(tc.tile_pool(name="vhp", bufs=3))
        expp = ctx.enter_context(tc.tile_pool(name="expp", bufs=4))
        atp = ctx.enter_context(tc.tile_pool(name="atp", bufs=TB))
        gelp = ctx.enter_context(tc.tile_pool(name="gelp", bufs=FB))
        qgp = ctx.enter_context(tc.tile_pool(name="qgp", bufs=FB))
        drp = ctx.enter_context(tc.tile_pool(name="drp", bufs=4, space="DRAM"))
        outp = ctx.enter_context(tc.tile_pool(name="outp", bufs=4))
        wop = ctx.enter_context(tc.tile_pool(name="wop", bufs=3))
        ps = ctx.enter_context(tc.tile_pool(name="ps", bufs=5, space="PSUM"))
        pst_p = ctx.enter_context(tc.tile_pool(name="pst", bufs=1, space="PSUM"))
        pso = ctx.enter_context(tc.tile_pool(name="pso", bufs=2, space="PSUM"))

        # ---------------- constants ----------------
        tri = const.tile([P, P], F32)
        nc.sync.dma_start(out=tri[:], in_=tri_in[:])
        eps_c = const.tile([P, 1], F32)
        nc.vector.memset(eps_c[:], EPS)
        ones_row = const.tile([1, P], F32)
        nc.vector.memset(ones_row[:], 1.0)
        ones_col = const.tile([P, 1], F32)
        nc.vector.memset(ones_col[:], 1.0)
        ident_bf = const.tile([P, P], BF16)
        make_identity(nc, ident_bf)

        # ---------------- load x into resident h tiles ----------------
        h = []
        for tb in range(TB):
            ht = hp.tile([P, D], F32, tag="h")
            nc.sync.dma_start(out=ht[:], in_=x_in[tb * P:(tb + 1) * P, :])
            h.append(ht)

        # =========== helpers ===========
        def cross_part_absmax(tiles, tag):
            """tiles: list of [128, n] f32 -> [128,1] all-partition absmax
            (batch-local global max), clamped at 1e-8."""
            acc = None
            for tl in tiles:
                c = colp.tile([P, 1], F32, tag="col", bufs=12)
                nc.vector.tensor_reduce(out=c[:], in_=tl[:], axis=AX.X,
                                        op=ALU.max, apply_absolute_value=True)
                if acc is None:
                    acc = c
                else:
                    nxt = colp.tile([P, 1], F32, tag="col", bufs=12)
                    nc.vector.tensor_max(out=nxt[:], in0=acc[:], in1=c[:])
                    acc = nxt
            m1 = stat.tile([1, 1], F32, tag="cpm1")
            nc.gpsimd.tensor_reduce(out=m1[:], in_=acc[:], axis=AX.C, op=ALU.max)
            mb = colp.tile([P, 1], F32, tag="mhat", bufs=4)
            nc.gpsimd.partition_broadcast(mb[:], m1[:], channels=P)
            mc = colp.tile([P, 1], F32, tag="mhat", bufs=4)
            nc.vector.tensor_scalar_max(out=mc[:], in0=mb[:], scalar1=1e-8)
            return mc

        def iscale_of(mhat, tag):
            # returns INVERSE scale 127/m (quantize multiplies by this)
            rcp = colp.tile([P, 1], F32, tag="isc", bufs=6)
            nc.vector.reciprocal(out=rcp[:], in_=mhat[:])
            inv = colp.tile([P, 1], F32, tag="isc", bufs=6)
            nc.vector.tensor_scalar_mul(out=inv[:], in0=rcp[:], scalar1=127.0)
            return inv

        def quantize_tiles(src_tiles, isc, n=D):
            """f32 [128,n] tiles -> bf16 [128,n] tiles (int8 round/saturate)."""
            res = []
            for st in src_tiles:
                q8 = qxp.tile([P, T], I8, tag="q8")
                nc.vector.tensor_scalar_mul(out=q8[:, :n], in0=st[:], scalar1=isc[:])
                qb = qtp.tile([P, T], BF16, tag="qt")
                nc.vector.tensor_copy(out=qb[:, :n], in_=q8[:, :n])
                res.append(qb)
            return res

        def transpose_to(qtiles):
            """TB x [128, D] bf16 (rows=t) -> DB x [128, T] bf16 (rows=d)."""
            outt = [qtp.tile([P, T], BF16, tag="qt", name="qt") for _ in range(DB)]
            for tb in range(TB):
                for db in range(DB):
                    pst = pst_p.tile([P, P], BF16, tag="pstr", name="pstr")
                    nc.tensor.transpose(pst[:P, :P],
                                        qtiles[tb][:, db * P:(db + 1) * P],
                                        ident_bf[:])
                    nc.vector.tensor_copy(out=outt[db][:, tb * P:(tb + 1) * P],
                                          in_=pst[:P, :P])
            return outt

        def layer_norm(tiles):
            outs = []
            for tb in range(TB):
                st6 = stat.tile([P, 6], F32, tag="bn6")
                nc.vector.bn_stats(out=st6[:], in_=tiles[tb][:])
                mv = stat.tile([P, 2], F32, tag="bn2")
                nc.vector.bn_aggr(out=mv[:], in_=st6[:])
                std = colp.tile([P, 1], F32, tag="col", bufs=12)
                nc.scalar.activation(out=std[:], in_=mv[:, 1:2], func=ACTF.Sqrt,
                                     bias=eps_c[:], scale=1.0)
                rstd = colp.tile([P, 1], F32, tag="col", bufs=12)
                nc.vector.reciprocal(out=rstd[:], in_=std[:])
                ot = lnp.tile([P, D], F32, tag="ln")
                nc.vector.tensor_scalar(out=ot[:], in0=tiles[tb][:],
                                        scalar1=mv[:, 0:1], scalar2=rstd[:],
                                        op0=ALU.subtract, op1=ALU.mult)
                outs.append(ot)
            return outs

        # ---------------- ctx prep (encoder mean + quantize, once) ----------
        ctx_rows = []
        for b in range(B):
            pctx = ps.tile([P, VN], F32, tag="psum")
            for sb in range(S // P):
                et = lnp.tile([P, D], F32, tag="ln")
                nc.sync.dma_start(
                    out=et[:], in_=enc_in[b * S + sb * P: b * S + (sb + 1) * P, :])
                nc.tensor.matmul(pctx[:1, :D], ones_col[:], et[:],
                                 start=(sb == 0), stop=(sb == S // P - 1))
            cr = stat.tile([1, D], F32, tag="ctxr", bufs=B)
            nc.scalar.activation(out=cr[:], in_=pctx[:1, :D],
                                 func=ACTF.Copy, scale=1.0 / S)
            ctx_rows.append(cr)
        cacc = None
        for b in range(B):
            cm = stat.tile([1, 1], F32, tag="cm", bufs=4)
            nc.vector.tensor_reduce(out=cm[:], in_=ctx_rows[b][:], axis=AX.X,
                                    op=ALU.max, apply_absolute_value=True)
            if cacc is None:
                cacc = cm
            else:
                nx = stat.tile([1, 1], F32, tag="cm", bufs=4)
                nc.vector.tensor_max(out=nx[:], in0=cacc[:], in1=cm[:])
                cacc = nx
        cmb = colp.tile([P, 1], F32, tag="ctxm")
        nc.gpsimd.partition_broadcast(cmb[:], cacc[:], channels=P)
        cmc = colp.tile([P, 1], F32, tag="ctxm")
        nc.vector.tensor_scalar_max(out=cmc[:], in0=cmb[:], scalar1=1e-8)
        isc_ctx = iscale_of(cmc, "ctx")
        q8row = stat.tile([1, D], I8, tag="q8ctx")
        nc.vector.tensor_scalar_mul(out=q8row[:], in0=ctx_rows[0][:],
                                    scalar1=isc_ctx[:1, :])
        dctx8 = drp.tile([1, D], I8, tag="dctx8")
        nc.sync.dma_start(out=dctx8[:], in_=q8row[:])
        qctx8 = stat.tile([P, DB], I8, tag="qctx8")
        nc.sync.dma_start(out=qctx8[:],
                          in_=dctx8[:].rearrange("one (j p) -> p (one j)", p=P))
        qctxT = const.tile([P, DB], BF16)
        nc.scalar.copy(out=qctxT[:], in_=qctx8[:])

        # ---------------- transformer layers ----------------
        for l in range(L):
            wsq, wsk, wsv, wso, wse, ws1, ws2 = ws[l]

            wq_sb = [wsp.tile([P, D], BF16, tag="ws", name="ws") for _ in range(DB)]
            wk_sb = [wsp.tile([P, D], BF16, tag="ws", name="ws") for _ in range(DB)]
            wv_sb = [wsp.tile([P, D], BF16, tag="ws", name="ws") for _ in range(DB)]
            for db in range(DB):
                nc.sync.dma_start(out=wq_sb[db][:], in_=wq[l][db * P:(db + 1) * P, :])
                nc.sync.dma_start(out=wk_sb[db][:], in_=wk[l][db * P:(db + 1) * P, :])
                nc.sync.dma_start(out=wv_sb[db][:], in_=wv[l][db * P:(db + 1) * P, :])
            w1_sb = [w1p.tile([P, FF], BF16, tag="w1", name="w1") for _ in range(DB)]
            for db in range(DB):
                nc.sync.dma_start(out=w1_sb[db][:], in_=w1[l][db * P:(db + 1) * P, :])
            bq_sb = const.tile([P, DB], F32, tag="bqc")
            bk_sb = const.tile([P, DB], F32, tag="bkc")
            nc.sync.dma_start(out=bq_sb[:], in_=bq_t[l][:])
            nc.sync.dma_start(out=bk_sb[:], in_=bk_t[l][:])

            # --- ln1 + quantize + transpose ---
            ln1 = layer_norm(h)
            if dbg and l == 0:
                for tb in range(TB):
                    nc.sync.dma_start(out=dbg["dbg_ln1"][tb * P:(tb + 1) * P, :],
                                      in_=ln1[tb][:])
            m_ln1 = cross_part_absmax(ln1, f"ln1_{l}")
            isc1 = iscale_of(m_ln1, f"ln1_{l}")
            q1 = quantize_tiles(ln1, isc1)
            q1T = transpose_to(q1)

            # --- qkv matmuls ---
            sc_q = colp.tile([P, 1], F32, tag="sc", bufs=6)
            nc.vector.tensor_scalar_mul(out=sc_q[:], in0=m_ln1[:], scalar1=wsq * INV_SQRT_HD / 127.0)
            sc_k = colp.tile([P, 1], F32, tag="sc", bufs=6)
            nc.vector.tensor_scalar_mul(out=sc_k[:], in0=m_ln1[:], scalar1=wsk / 127.0)
            sc_v = colp.tile([P, 1], F32, tag="sc", bufs=6)
            nc.vector.tensor_scalar_mul(out=sc_v[:], in0=m_ln1[:], scalar1=wsv / 127.0)
            qT, kT = [], []
            for ob in range(DB):
                pq = ps.tile([P, VN], F32, tag="psum")
                for db in range(DB):
                    nc.tensor.matmul(pq[:, :T], wq_sb[db][:, ob * P:(ob + 1) * P],
                                     q1T[db][:], start=(db == 0), stop=(db == DB - 1))
                qf = qkp.tile([P, T], F32, tag="qk")
                nc.scalar.activation(out=qf[:], in_=pq[:, :T], func=ACTF.Identity,
                                     bias=bq_sb[:, ob:ob + 1], scale=sc_q[:])
                qT.append(qf)
                pk = ps.tile([P, VN], F32, tag="psum")
                for db in range(DB):
                    nc.tensor.matmul(pk[:, :T], wk_sb[db][:, ob * P:(ob + 1) * P],
                                     q1T[db][:], start=(db == 0), stop=(db == DB - 1))
                kf = qkp.tile([P, T], F32, tag="qk")
                nc.scalar.activation(out=kf[:], in_=pk[:, :T], func=ACTF.Identity,
                                     bias=bk_sb[:, ob:ob + 1], scale=sc_k[:])
                kT.append(kf)
            if dbg and l == 0:
                for ob in range(DB):
                    nc.sync.dma_start(out=dbg["dbg_qT"][ob * P:(ob + 1) * P, :],
                                      in_=qT[ob][:])
            v_sb = []
            for tb in range(TB):
                pv = ps.tile([P, VN], F32, tag="psum")
                for db in range(DB):
                    nc.tensor.matmul(pv[:, :D], q1T[db][:, tb * P:(tb + 1) * P],
                                     wv_sb[db][:], start=(db == 0), stop=(db == DB - 1))
                vt = vp.tile([P, H, HD + 1], F32, tag="v")
                nc.scalar.activation(
                    out=vt[:, :, :HD],
                    in_=pv[:, :D].rearrange("p (h d) -> p h d", h=H),
                    func=ACTF.Identity, bias=0.0, scale=sc_v[:])
                nc.vector.memset(vt[:, :, HD:HD + 1], 1.0)
                v_sb.append(vt)

            # --- attention (fp32) ---
            attnT = [atp.tile([P, T], F32, tag="at", name="at") for _ in range(DB)]
            for hh in range(H):
                ob, off = hh // 2, (hh % 2) * HD
                po = pso.tile([P, VN], F32, tag="psumo")
                for kb in range(TB):
                    qoff = kb * P
                    pss = ps.tile([P, VN], F32, tag="psum")
                    nc.tensor.matmul(
                        pss[:, qoff:T],
                        kT[ob][off:off + HD, kb * P:(kb + 1) * P],
                        qT[ob][off:off + HD, qoff:T],
                        start=True, stop=True)
                    mrow = colp.tile([P, 1], F32, tag="mrow", bufs=6)
                    nc.vector.tensor_reduce(out=mrow[:], in_=pss[:, qoff:T],
                                            axis=AX.X, op=ALU.max)
                    nmrow = colp.tile([P, 1], F32, tag="mrow", bufs=6)
                    nc.vector.tensor_scalar_mul(out=nmrow[:], in0=mrow[:], scalar1=-1.0)
                    emrow = colp.tile([P, 1], F32, tag="mrow", bufs=6)
                    nc.scalar.activation(out=emrow[:], in_=mrow[:], func=ACTF.Exp)
                    ex = expp.tile([P, T], F32, tag="exp")
                    if qoff:
                        nc.gpsimd.memset(ex[:, :qoff], 0.0)
                    nc.scalar.activation(out=ex[:, qoff:T], in_=pss[:, qoff:T],
                                         func=ACTF.Exp, bias=nmrow[:], scale=1.0)
                    nc.vector.tensor_mul(out=ex[:, qoff:qoff + P],
                                         in0=ex[:, qoff:qoff + P], in1=tri[:])
                    vh = vhp.tile([P, HD + 1], F32, tag="vh")
                    nc.vector.tensor_scalar_mul(
                        out=vh[:], in0=v_sb[kb][:, hh, :], scalar1=emrow[:])
                    nc.tensor.matmul(po[:HD + 1, qoff:T], vh[:], ex[:, qoff:T],
                                     start=(kb == 0), stop=(kb == TB - 1))
                rec = stat.tile([1, T], F32, tag="rec")
                nc.vector.reciprocal(out=rec[:], in_=po[HD:HD + 1, :T])
                pb = ps.tile([P, VN], F32, tag="psum")
                nc.tensor.matmul(pb[:HD, :T], ones_row[:1, :HD], rec[:],
                                 start=True, stop=True)
                nc.scalar.copy(out=attnT[ob][off:off + HD, :], in_=po[:HD, :T])
                nc.vector.tensor_mul(out=attnT[ob][off:off + HD, :],
                                     in0=attnT[ob][off:off + HD, :],
                                     in1=pb[:HD, :T])
            if dbg and l == 0:
                for ob in range(DB):
                    nc.sync.dma_start(out=dbg["dbg_attn"][ob * P:(ob + 1) * P, :],
                                      in_=attnT[ob][:])

            # --- attention output projection (Wo) + residual ---
            wo_sb = [wsp.tile([P, D], BF16, tag="ws", name="ws") for _ in range(DB)]
            we_sb = [wsp.tile([P, D], BF16, tag="ws", name="ws") for _ in range(DB)]
            for db in range(DB):
                nc.sync.dma_start(out=wo_sb[db][:], in_=wo[l][db * P:(db + 1) * P, :])
                nc.sync.dma_start(out=we_sb[db][:], in_=wenc[l][db * P:(db + 1) * P, :])
            m_o = cross_part_absmax(attnT, f"o_{l}")
            isc_o = iscale_of(m_o, f"o_{l}")
            qo = quantize_tiles(attnT, isc_o, n=T)
            sc_o = colp.tile([P, 1], F32, tag="sc", bufs=6)
            nc.vector.tensor_scalar_mul(out=sc_o[:], in0=m_o[:], scalar1=wso / 127.0)
            for tb in range(TB):
                pw = ps.tile([P, VN], F32, tag="psum")
                for ob in range(DB):
                    nc.tensor.matmul(pw[:, :D], qo[ob][:, tb * P:(tb + 1) * P],
                                     wo_sb[ob][:], start=(ob == 0), stop=(ob == DB - 1))
                nc.vector.scalar_tensor_tensor(out=h[tb][:], in0=pw[:, :D],
                                               scalar=sc_o[:], in1=h[tb][:],
                                               op0=ALU.mult, op1=ALU.add)

            # --- encoder-context projection + residual (broadcast over t) ---
            pe = ps.tile([P, VN], F32, tag="psum")
            for db in range(DB):
                nc.tensor.matmul(pe[:1, :D], qctxT[:, db:db + 1], we_sb[db][:],
                                 start=(db == 0), stop=(db == DB - 1))
            enc_row = stat.tile([1, D], F32, tag="encrow", bufs=1)
            sc_e = stat.tile([1, 1], F32, tag="sc_e")
            nc.scalar.mul(out=sc_e[:], in_=cmc[:1, :], mul=wse / 127.0)
            nc.scalar.activation(out=enc_row[:], in_=pe[:1, :D],
                                 func=ACTF.Copy, scale=sc_e[:])
            pbe = ps.tile([P, VN], F32, tag="psum")
            nc.tensor.matmul(pbe[:, :D], ones_row[:1, :P], enc_row[:],
                             start=True, stop=True)
            for tb in range(TB):
                nc.vector.tensor_add(out=h[tb][:], in0=h[tb][:], in1=pbe[:, :D])

            # --- FFN ---
            ln3 = layer_norm(h)
            m_ln3 = cross_part_absmax(ln3, f"ln3_{l}")
            isc3 = iscale_of(m_ln3, f"ln3_{l}")
            q3 = quantize_tiles(ln3, isc3)
            q3T = transpose_to(q3)
            b1_sb = const.tile([P, FB], F32, tag="b1c")
            nc.sync.dma_start(out=b1_sb[:], in_=b1_t[l][:])
            sc_1 = colp.tile([P, 1], F32, tag="sc", bufs=6)
            nc.vector.tensor_scalar_mul(out=sc_1[:], in0=m_ln3[:], scalar1=ws1 / 127.0)
            gel = []
            for fb in range(FB):
                pf = ps.tile([P, VN], F32, tag="psum")
                for db in range(DB):
                    nc.tensor.matmul(pf[:, :T], w1_sb[db][:, fb * P:(fb + 1) * P],
                                     q3T[db][:], start=(db == 0), stop=(db == DB - 1))
                gt = gelp.tile([P, T], F32, tag="gel")
                nc.scalar.activation(out=gt[:], in_=pf[:, :T], func=ACTF.Gelu,
                                     bias=b1_sb[:, fb:fb + 1], scale=sc_1[:])
                gel.append(gt)
            if dbg and l == 0:
                for fb in range(FB):
                    nc.sync.dma_start(out=dbg["dbg_gelu"][fb * P:(fb + 1) * P, :],
                                      in_=gel[fb][:])
            m_g = cross_part_absmax(gel, f"g_{l}")
            isc_g = iscale_of(m_g, f"g_{l}")
            sc_2 = colp.tile([P, 1], F32, tag="sc", bufs=6)
            nc.vector.tensor_scalar_mul(out=sc_2[:], in0=m_g[:], scalar1=ws2 / 127.0)
            qg = []
            for fb in range(FB):
                g8 = qxp.tile([P, T], I8, tag="q8")
                nc.vector.tensor_scalar_mul(out=g8[:], in0=gel[fb][:], scalar1=isc_g[:])
                gq = qgp.tile([P, T], BF16, tag="qg")
                eng_up = nc.vector if fb % 2 == 0 else nc.gpsimd
                eng_up.tensor_copy(out=gq[:], in_=g8[:])
                qg.append(gq)
            pf2 = [ps.tile([P, VN], F32, tag="psum", name="pf2") for _ in range(TB)]
            for fb in range(FB):
                w2t = w2p.tile([P, D], BF16, tag="w2", name="w2", bufs=6)
                nc.sync.dma_start(out=w2t[:], in_=w2[l][fb * P:(fb + 1) * P, :])
                for tb in range(TB):
                    nc.tensor.matmul(pf2[tb][:, :D], qg[fb][:, tb * P:(tb + 1) * P],
                                     w2t[:], start=(fb == 0), stop=(fb == FB - 1))
            for tb in range(TB):
                nc.vector.scalar_tensor_tensor(out=h[tb][:], in0=pf2[tb][:, :D],
                                               scalar=sc_2[:], in1=h[tb][:],
                                               op0=ALU.mult, op1=ALU.add)
            if dbg:
                tgt = dbg["dbg_h0"] if l == 0 else dbg["dbg_h1"]
                for tb in range(TB):
                    nc.sync.dma_start(out=tgt[tb * P:(tb + 1) * P, :], in_=h[tb][:])

        # ---------------- final LN + output projection ----------------
        ws_out = ws[L]
        lnf = layer_norm(h)
        m_h = cross_part_absmax(lnf, "lnf")
        isc_h = iscale_of(m_h, "lnf")
        qh = quantize_tiles(lnf, isc_h)
        qhT = transpose_to(qh)
        sc_out = colp.tile([P, 1], F32, tag="sc_out")
        nc.scalar.mul(out=sc_out[:], in_=m_h[:], mul=ws_out / 127.0)
        nvb = VBF + (1 if VREM else 0)
        # process vocab blocks in pairs: one [128, 1024] bf16 staging tile per
        # (tb, pair) -> bigger, fewer output DMAs
        pair_starts = list(range(0, VBF - 1, 2))       # (0,1), (2,3), ... (60,61)
        tail = [VBF] if VREM else []                   # remainder block alone
        evict_i = 0
        for pv, v0 in enumerate(pair_starts + tail):
            pair = v0 < VBF - 1
            vws = [(v0, VN), (v0 + 1, VN)] if pair else [(v0, VREM)]
            w8 = [wop.tile([P, 2 * VN], I8, tag="w8", name="w8", bufs=5)
                  for _ in range(DB)]
            wtile = [wop.tile([P, 2 * VN], BF16, tag="wout", name="wout", bufs=5)
                     for _ in range(DB)]
            wn_tot = sum(vn for _, vn in vws)
            for db in range(DB):
                nc.sync.dma_start(out=w8[db][:, :wn_tot],
                                    in_=wout_in[db * P:(db + 1) * P,
                                                v0 * VN:v0 * VN + wn_tot])
                eng_up = nc.gpsimd if db % 2 == 0 else nc.vector
                eng_up.tensor_copy(out=wtile[db][:, :wn_tot],
                                   in_=w8[db][:, :wn_tot])
            for tb in range(TB):
                lt = outp.tile([P, 2 * VN], BF16, tag="lt")
                for si, (vb, vn) in enumerate(vws):
                    pl = ps.tile([P, VN], F32, tag="psum")
                    for db in range(DB):
                        nc.tensor.matmul(
                            pl[:, :vn],
                            qhT[db][:, tb * P:(tb + 1) * P],
                            wtile[db][:, si * VN:si * VN + vn],
                            start=(db == 0), stop=(db == DB - 1))
                    if evict_i % 2 == 0:
                        nc.scalar.activation(out=lt[:, si * VN:si * VN + vn],
                                             in_=pl[:, :vn], func=ACTF.Copy,
                                             scale=sc_out[:])
                    else:
                        nc.vector.tensor_scalar_mul(
                            out=lt[:, si * VN:si * VN + vn], in0=pl[:, :vn],
                            scalar1=sc_out[:])
                    evict_i += 1
                eng = nc.sync if (tb % 2 == 0) else nc.scalar
                eng.dma_start(
                    out=out[tb * P:(tb + 1) * P, v0 * VN:v0 * VN + wn_tot],
                    in_=lt[:, :wn_tot])


def _prep(inputs):
    """Host-side packing shared across cores; returns (common dict, wscales)."""
    f32 = np.float32
    ws = []
    common = {}
    for l in range(L):
        packed = []
        for name, W in [("wq", inputs["Wq"][l]), ("wk", inputs["Wk"][l]),
                        ("wv", inputs["Wv"][l]), ("wo", inputs["Wo"][l]),
                        ("wenc", inputs["Wenc"][l]), ("w1", inputs["W1"][l]),
                        ("w2", inputs["W2"][l])]:
            qWT, s = _quantize_weight(W)
            common[f"{name}{l}"] = qWT
            packed.append(s)
        ws.append(tuple(packed))
        common[f"bq{l}"] = _col_layout(
            np.asarray(inputs["bq"][l], f32) * f32(INV_SQRT_HD))
        common[f"bk{l}"] = _col_layout(inputs["bk"][l])
        common[f"b1{l}"] = _col_layout(inputs["b1"][l])
    qWoutT, s_out = _quantize_weight(inputs["Wout"])
    common["wout"] = np.ascontiguousarray(
        np.asarray(qWoutT, np.float32).astype(np.int8))
    ws.append(s_out)
    common["tri"] = np.triu(np.ones((P, P), dtype=f32))
    return common, ws


def kernel(**inputs):
    debug = bool(int(os.environ.get("BITGEN_DEBUG", "0")))
    common, ws = _prep(inputs)

    # Fast path assumes the model's declared fills: zero biases on the layers
    # without per-partition bias layout, identity LN affines.
    for nm in ["bo", "benc", "b2", "bout", "bv",
               "ln1b", "ln2b", "ln3b", "lnfb"]:
        assert not np.any(np.asarray(inputs[nm])), f"nonzero {nm} unsupported"
    for nm in ["ln1g", "ln2g", "ln3g", "lnfg"]:
        assert np.all(np.asarray(inputs[nm]) == 1.0), f"non-unit {nm} unsupported"

    key = ("v1", debug, tuple(np.asarray(w, np.float64).tobytes()
                              for w in (tuple(ws[l]) for l in range(L)))), ws[L]
    key = (repr(ws), debug)
    if key not in _NC_CACHE:
        _NC_CACHE[key] = build(ws, debug=debug)
    nc = _NC_CACHE[key]

    enc = np.asarray(inputs["encoder_output"], np.float32)
    x = np.asarray(inputs["x"], np.float32)
    in_maps = []
    for c in range(NCORES):
        m = dict(common)
        m["x"] = np.ascontiguousarray(x[c])
        # roll so this core's batch is the first S-row block (the kernel
        # quantizes ctx row 0 as its own batch, using all rows for the scale)
        m["enc"] = np.ascontiguousarray(
            np.roll(enc, -c, axis=0).reshape(B * S, D))
        in_maps.append(m)

    res = run_bass_kernel_spmd(nc, in_maps, list(range(NCORES)))
    outs = [res.results[c]["logits"].astype(np.float32) for c in range(NCORES)]
    return np.stack(outs, axis=0)
